# revision 114
# baseline (speedup 1.0000x reference)
"""Trainium2 Bass kernel for a GQA attention block (B=2, L=2048, D=2048,
16 q-heads / 8 kv-heads, head_dim=128), sharded over 8 NeuronCores.

Sharding: core c -> batch b = c // 4, head-group g = c % 4 (4 q-heads and
their 2 kv-heads).  Each core computes its heads' attention plus the partial
output projection; the host sums the 4 partials per batch.

Self-contained: only needs numpy / ml_dtypes / concourse (on PYTHONPATH in
this container).
"""

import math
import sys

for _p in ("/root/.axon_site", "/root/.axon_site/_ro/trn_rl_repo",
           "/root/.axon_site/_ro/pypackages"):
    if _p not in sys.path:
        sys.path.append(_p)

import numpy as np
import ml_dtypes

import concourse.bass as bass
import concourse.bass2jax as bass2jax
import concourse.bass_isa as bass_isa
import concourse.mybir as mybir
import concourse.tile as tile
from concourse.masks import make_identity
from concourse.bass_utils import run_bass_kernel_spmd
from concourse.vector_clock import ScopedClock, VectorClock


def _legalize_bir_waits(bir_bytes):
    """This walrus build supports only ONE sync-wait slot per instruction.
    Hoist extra waits onto NoOp instructions inserted just before the
    offender (same engine, so the engine stream still blocks in order)."""
    import orjson

    d = orjson.loads(bir_bytes)
    n_split = 0
    for f in d["functions"]:
        for bb in f["blocks"]:
            out = []
            for inst in bb["instructions"]:
                si = inst.get("sync_info")
                waits = (si or {}).get("on_wait") or []
                if len(waits) > 1:
                    for j, w in enumerate(waits[:-1]):
                        n_split += 1
                        out.append({
                            "engine": inst["engine"], "ins": [], "outs": [],
                            "name": f"{inst['name']}__w{j}",
                            "opcode": "NoOp",
                            "sync_info": {"on_wait": [w], "on_update": []},
                        })
                    si["on_wait"] = [waits[-1]]
                out.append(inst)
            bb["instructions"] = out
    return orjson.dumps(d)


_orig_compile_bir_kernel = bass2jax.compile_bir_kernel


def _patched_compile_bir_kernel(ant_bir_str, *args, **kwargs):
    return _orig_compile_bir_kernel(_legalize_bir_waits(ant_bir_str), *args, **kwargs)


bass2jax.compile_bir_kernel = _patched_compile_bir_kernel

BF16 = mybir.dt.bfloat16
F32 = mybir.dt.float32

# Full-problem constants
B, L, D = 2, 2048, 2048
N_HEADS, N_KV, H = 16, 8, 128
EPS = 1e-6
ROPE_THETA = 1e6
N_CORES = 8
QH_PER_CORE = N_HEADS // (N_CORES // B)   # 4
KV_PER_CORE = N_KV // (N_CORES // B)      # 2
SCALE = H ** -0.5


class PatchedTileContext(tile.TileContext):
    """This walrus build only supports one sync-wait slot on a CTRL (Drain)
    instruction; split the tail-drain waits across one drain per processor."""

    def _drain_and_barrier(self, tick_clock, wait_clock):
        gc = tick_clock.global_clock
        n = len(gc)
        for p in range(n):
            t = gc[p]
            if t > 0:
                vc = VectorClock([t if i == p else 0 for i in range(n)])
                d = self.nc.sync.drain()
                wait_clock.add_sem_waits(d.ins, ScopedClock({None: vc}))
                si = d.ins.sync_info
                nw = len(si.on_wait) if si is not None else 0
                assert nw <= 1, f"proc {p} produced {nw} waits"
        self.nc.all_engine_barrier()
        assert self.sems is not None
        popped = self.nc._tile_sem_poison_stack.pop()
        assert popped is self._sem_poison
        self.nc.clear_and_free_semaphores(list(self.sems.allocated().values()))
        self.nc.all_engine_barrier()


# engine assignment knobs (tuned against the cost-model timeline)
TCOPY_ENG = "scalar"   # transpose PSUM->SBUF copies (gpsimd cannot touch PSUM)
VCOPY_ENG = "vector"  # v PSUM->SBUF copy
OCOPY_SPLIT = True     # split phase-D copies between DVE and ACT
DELAY_T = True         # transpose qkn one L-block late
ROPE_STT = True        # fold rstd into stt ops reading the SBUF staging
WQKV_SPLIT = True      # split the wqkv load into 4 chunks after xt(0)
WO_LATE = True         # load wo just before it's needed
INTERLEAVE = "fine"    # "fine": B/C head interleave + C/D unit interleave
PE_WARMUP = 0
QKRAW_ENG = "scalar"   # staging copies of q+k PSUM->SBUF
C_DEPTH = 5            # attention chunk software-pipeline depth
EXP_BUFS = 7
ROPE_K_ENG = "gpsimd"  # engine for k-head rope multiplies
WQKV_FIRST1 = False    # make the first wqkv chunk a single dc
STATS_BUFS = 6
WORK_BUFS = 3
SCORES_BUFS = 3        # scores PSUM ring (av 3 + scores 3 + proj 2 = 8 banks)
DEN_MODE = "pool"      # "pool": gpsimd C-axis reduce; "mm": PE ones-matmul


def _copy(nc, eng, out, in_):
    if eng == "vector":
        nc.vector.tensor_copy(out=out, in_=in_)
    elif eng == "gpsimd":
        nc.gpsimd.tensor_copy(out=out, in_=in_)
    else:
        nc.scalar.copy(out=out, in_=in_)


def build_core_kernel(L_=L, D_=D, nq=QH_PER_CORE, nkv=KV_PER_CORE, causal=True):
    """One core's program.  Inputs (DRAM):
      xT    [L/128, 128, D] bf16 — host-preblocked x (see _x_block)
      wqkv  [D, nq*H + 2*nkv*H] bf16  ([wq heads | wk heads | wv heads])
      wo    [nq*H, D] bf16
      rope  [L/128/G, 128, G*8*(H/2)] f32 — preblocked A,B,C,D cos/sin
            tables for q then k, norm weights folded in (see _rope_block)
      maskT [L, L] bf16 (only if causal=False; 0/1 multiplicative, [s, l])
    Output:
      out [L, D] f32 — partial sum over this core's heads.

    Layout strategy: projections produce q/k/v in natural [L-part, H] layout
    (RMS-norm + RoPE are row-wise there, all bf16 so DVE gets its 2x/4x
    modes; k-head rope multiplies run on gpsimd), q/k are transposed per
    128-block on the PE (identity matmul); attention runs fully transposed —
    scores^T = kT^T qT, exp on ACT, causal mask via a 128-col affine_select
    (gpsimd) or triangle multiply (DVE), av^T accumulated over S-chunks in
    PSUM — so av^T feeds the output projection as lhsT with zero further
    transposes.  The softmax denominator is a bf16 running sum of the exp
    tiles on DVE, reduced across partitions once per (head, q-group) by a
    gpsimd C-axis tensor_reduce, then broadcast by a tiny f32r PE matmul;
    the whole finalize is deferred into the next head's chunk stream.
    Schedule: dc-interleaved fused start for blocks 0-2 chasing the
    dc-ordered wqkv/x DMA queue, C0/C1 head-units between the later B
    blocks, output-projection (lb,db)-units woven into C2/C3's chunk
    streams, and an hh-outer fused final D group.
    """
    HH = H // 2
    n_lb = L_ // 128          # L blocks of 128
    n_dc = D_ // 128          # D contraction chunks
    n_lqb = L_ // 512         # q blocks of 512
    QCOLS = nq * H
    KCOLS = nkv * H
    KV_COLS = 2 * nkv * H
    W_COLS = QCOLS + KV_COLS
    assert W_COLS % 512 == 0
    n_wslab = W_COLS // 512   # 512-wide slabs of the qkv projection

    nc = bass.Bass()
    # x, host-preblocked: [lb, p, dc*128] with element = x[lb*128+l, dc*128+p]
    xT_d = nc.dram_tensor("xT", [L_ // 128, 128, D_], BF16, kind="ExternalInput")
    wqkv_d = nc.dram_tensor("wqkv", [D_, W_COLS], BF16, kind="ExternalInput")
    wo_d = nc.dram_tensor("wo", [QCOLS, D_], BF16, kind="ExternalInput")
    # rope tables, host-preblocked: [group, p, (lb-in-group, table 0..7, j)]
    ROPE_GRP = 4 if (L_ // 128) % 4 == 0 else 1
    rope_d = nc.dram_tensor(
        "rope", [L_ // 128 // ROPE_GRP, 128, ROPE_GRP * 8 * HH], BF16,
        kind="ExternalInput",
    )
    if not causal:
        maskT_d = nc.dram_tensor("maskT", [L_, L_], BF16, kind="ExternalInput")
    out_d = nc.dram_tensor("out", [L_, D_], F32, kind="ExternalOutput")
    nqk = nq + nkv  # q heads then k heads in the combined transposed tile

    with PatchedTileContext(nc) as tc:
        with (
            tc.tile_pool(name="res", bufs=1) as res,
            tc.tile_pool(name="ropetab", bufs=3) as ropetab,
            tc.tile_pool(name="work", bufs=WORK_BUFS) as work,
            tc.tile_pool(name="stats", bufs=STATS_BUFS) as stats,
            tc.tile_pool(name="expp", bufs=EXP_BUFS) as expp,
            tc.tile_pool(name="outp", bufs=3) as outp,
            tc.tile_pool(name="psum", bufs=1, space="PSUM") as psum,
            tc.tile_pool(name="maskp", bufs=2) as maskp,
        ):
            # ---- resident loads ----
            wqkv_sb = res.tile([128, n_dc, W_COLS], BF16, tag="wqkv")
            wqkv_r = wqkv_d.rearrange("(dc p) c -> p dc c", p=128)
            wo_sb = res.tile([128, nq, D_], BF16, tag="wo")

            def load_wo():
                nc.scalar.dma_start(
                    out=wo_sb, in_=wo_d.rearrange("(hh p) d -> p hh d", p=128)
                )

            if not WO_LATE:
                load_wo()
            ones_sb = res.tile([128, 1], BF16, tag="ones")
            nc.vector.memset(ones_sb, 1.0)
            # f32r ones row: the partition-broadcast matmul runs at bf16
            # speed for N>=256 while keeping full fp32 mantissa bits
            F32R = mybir.dt.float32r
            onesf_sb = res.tile([1, 128], F32R, tag="onesf")
            onesf_f32 = res.tile([1, 128], F32, tag="onesf32")
            nc.vector.memset(onesf_f32, 1.0)
            with nc.allow_low_precision(reason="f32r ones"):
                nc.vector.tensor_copy(out=onesf_sb, in_=onesf_f32)
            eps_sb = res.tile([128, 1], F32, tag="eps")
            nc.vector.memset(eps_sb, EPS)
            ident_sb = res.tile([128, 128], BF16, tag="ident")
            make_identity(nc, ident_sb)
            # 0/1 lower-triangle (keep f >= p) for the causal mask multiply
            tri_sb = res.tile([128, 128], BF16, tag="tri")
            nc.vector.memset(tri_sb, 1.0)
            nc.gpsimd.affine_select(
                out=tri_sb, in_=tri_sb, pattern=[[1, 128]],
                compare_op=mybir.AluOpType.is_ge, fill=0.0,
                base=0, channel_multiplier=-1,
            )
            if PE_WARMUP:
                pw = psum.tile([128, 128], BF16, tag="scores", bufs=3,
                               name="pe_warm")
                for _ in range(PE_WARMUP):
                    nc.tensor.transpose(pw, ident_sb, ident_sb)

            v_sb = res.tile([128, n_lb, KCOLS], BF16, tag="v")
            qkT_sb = res.tile([128, nqk, L_], BF16, tag="qkT")
            qkvT_sb = res.tile([128, nq, L_], BF16, tag="qkvT")

            # ---- phase B: qkv projection + rmsnorm + rope + transposes ----
            def head_stats(src, ssq6, h, eng="vector"):
                """sum(x^2) for one head: stt square with accumulator
                (x*1)*x; DVE gets it cheap in bf16, but the fused-start
                blocks use ACT Square to spare DVE's post-fused burst."""
                sq = work.tile([128, H], BF16, tag="sq")
                if eng == "act":
                    nc.scalar.activation(
                        out=sq, in_=src,
                        func=mybir.ActivationFunctionType.Square,
                        accum_out=ssq6[:, h:h + 1],
                    )
                else:
                    nc.vector.scalar_tensor_tensor(
                        out=sq, in0=src, scalar=1.0, in1=src,
                        op0=mybir.AluOpType.mult, op1=mybir.AluOpType.mult,
                        accum_out=ssq6[:, h:h + 1],
                    )

            def finish_stats(ssq6, rstd6):
                # one fused sqrt / reciprocal for all heads of the block
                nc.scalar.activation(
                    out=rstd6[:, 0:nqk], in_=ssq6[:, 0:nqk],
                    func=mybir.ActivationFunctionType.Sqrt,
                    bias=eps_sb, scale=1.0 / H,
                )
                nc.vector.reciprocal(out=rstd6[:, 0:nqk], in_=rstd6[:, 0:nqk])

            def rope_head(src, rstd, rtab, qkn, dcol):
                """RoPE one head (src: [128, H] bf16 in SBUF); bf16 for DVE
                2x/4x perf modes:
                h1 = (q1*rstd)*A - (q2*rstd)*B ; h2 = (q2*rstd)*C + (q1*rstd)*D
                k heads run their multiplies on gpsimd (plain tensor_tensor
                only -- Pool has no stt opcode), with the rstd prescale done
                on DVE where tensor_scalar gets the 4x mode."""
                qb = qkn[:, dcol * H:(dcol + 1) * H]
                mul = mybir.AluOpType.mult
                on_pool = dcol >= nq and ROPE_K_ENG == "gpsimd"
                veng = nc.gpsimd if on_pool else nc.vector
                if on_pool:
                    qn = work.tile([128, H], BF16, tag="qn")
                    nc.vector.tensor_scalar_mul(qn, src, rstd)
                    s1, s2 = qn[:, 0:HH], qn[:, HH:H]
                else:
                    s1, s2 = src[:, 0:HH], src[:, HH:H]
                # separate tile rings per engine: sharing them would make
                # DVE waits ride on Pool's slower ops via slot reuse
                tga, tgb = ("kt1", "kt2") if on_pool else ("t1", "t2")
                t1 = work.tile([128, HH], BF16, tag=tga, name="t1")
                t2 = work.tile([128, HH], BF16, tag=tgb, name="t2")

                def rmul(out, sx, tab):
                    if on_pool:
                        veng.tensor_mul(out, sx, tab)
                    else:
                        veng.scalar_tensor_tensor(
                            out=out, in0=sx, scalar=rstd, in1=tab,
                            op0=mul, op1=mul)

                rmul(t1, s1, rtab[:, 0, :])
                rmul(t2, s2, rtab[:, 1, :])
                veng.tensor_sub(qb[:, 0:HH], t1, t2)
                t3 = work.tile([128, HH], BF16, tag=tga, name="t3")
                t4 = work.tile([128, HH], BF16, tag=tgb, name="t4")
                rmul(t3, s2, rtab[:, 2, :])
                rmul(t4, s1, rtab[:, 3, :])
                veng.tensor_add(qb[:, HH:H], t3, t4)

            def transpose_block(qkn, lb):
                # transpose each head block on PE (identity matmul)
                for h in range(nqk):
                    pt = psum.tile([128, 128], BF16, tag="scores", bufs=SCORES_BUFS,
                                   name=f"pt_{lb}_{h}")
                    nc.tensor.transpose(
                        pt, qkn[:, h * H:(h + 1) * H], ident_sb
                    )
                    _copy(nc, TCOPY_ENG,
                          qkT_sb[:, h, lb * 128:(lb + 1) * 128], pt)

            state = {"pending": []}  # [(qkn, lb)] transposed T_DELAY late

            def preload_startup():
                """Interleave the wqkv chunks with xt pieces for blocks 0/1
                in dc order on ONE queue, so early proj matmuls are gated by
                the minimum prefix of bytes rather than whole-tensor DMAs."""
                if not (WQKV_SPLIT and n_dc >= 8):
                    nc.scalar.dma_start(out=wqkv_sb, in_=wqkv_r)
                    return {}
                xts = {}
                for i in range(min(4, n_lb)):
                    t = work.tile([128, n_dc, 128], BF16, tag="xt", bufs=4,
                                  name=f"xt_{i}")
                    xts[i] = t
                # weight chunks in dc order; xt pieces (>=2 dc so each DMA
                # descriptor stays >=512B) slotted between them
                # block 0's first piece goes down the SP queue so its DMA
                # latency chain overlaps the first weight chunk's
                nc.sync.dma_start(out=xts[0][:, 0:2, :],
                                  in_=xT_d[0, :, 0:256])
                wb = [0, 1, 2, 3, 4, 6, 8, 10, 12, 14, n_dc]
                xb = [0, 2, 4, 6, 8, 10, 12, 14, n_dc]
                xi = 0
                for i, j in zip(wb[:-1], wb[1:]):
                    nc.scalar.dma_start(
                        out=wqkv_sb[:, i:j, :], in_=wqkv_r[:, i:j, :]
                    )
                    while xi + 1 < len(xb) and xb[xi + 1] <= j:
                        a, b_ = xb[xi], xb[xi + 1]
                        for bi, t in xts.items():
                            if bi == 3 or (bi == 0 and a == 0):
                                continue  # block 3 loads after the hot path
                            nc.scalar.dma_start(
                                out=t[:, a:b_, :],
                                in_=xT_d[bi, :, a * 128:b_ * 128],
                            )
                        xi += 1
                if 3 in xts:
                    nc.scalar.dma_start(out=xts[3], in_=xT_d[3])
                return xts

            def b_fused_start(xts):
                """Projection matmuls for blocks 0..2 interleaved by dc so
                PE consumption tracks the dc-ordered DMA arrival; one PSUM
                accumulator pair per block drawn from the three tag rings."""
                tags = [("proj", 3), ("scores", SCORES_BUFS), ("av", 2)]
                pqs = {}
                for bi in xts:
                    tg, bf = tags[bi]
                    pqs[bi] = [psum.tile([128, 512], F32, tag=tg, bufs=bf,
                                         name=f"projf_{bi}_{s}")
                               for s in range(n_wslab)]
                for dc in range(n_dc):
                    for bi, t in xts.items():
                        for s in range(n_wslab):
                            nc.tensor.matmul(
                                pqs[bi][s],
                                t[:, dc, :],
                                wqkv_sb[:, dc, s * 512:(s + 1) * 512],
                                start=(dc == 0), stop=(dc == n_dc - 1),
                                skip_group_check=True,
                            )
                for bi in xts:
                    b_post(bi, pqs[bi])

            def b_block(lb, xt_pre=None):
                if xt_pre is not None:
                    xt = xt_pre
                else:
                    xt = work.tile([128, n_dc, 128], BF16, tag="xt", bufs=4,
                                   name=f"xt_{lb}")
                    nc.sync.dma_start(out=xt, in_=xT_d[lb])
                # dc-outer: both slabs accumulate in parallel, so early
                # weight chunks enable matmuls in arrival order
                pqs = [psum.tile([128, 512], F32, tag="proj", bufs=3,
                                 name=f"proj_{lb}_{s}")
                       for s in range(n_wslab)]
                for dc in range(n_dc):
                    for s in range(n_wslab):
                        nc.tensor.matmul(
                            pqs[s],
                            xt[:, dc, :],
                            wqkv_sb[:, dc, s * 512:(s + 1) * 512],
                            start=(dc == 0), stop=(dc == n_dc - 1),
                            skip_group_check=True,
                        )
                b_post(lb, pqs)

            def b_post(lb, pqs):
                # bulk-stage q+k to SBUF (bf16 so rope DVE ops get 2x/4x);
                # v goes straight to its resident tile
                qkraw = work.tile([128, QCOLS + KCOLS], BF16, tag="qkraw",
                                  bufs=3, name=f"qkraw_{lb}")
                off = 0
                for s in range(n_wslab):
                    w = min(512, QCOLS + KCOLS - off)
                    if w > 0:
                        _copy(nc, QKRAW_ENG,
                              qkraw[:, off:off + w], pqs[s][:, 0:w])
                    off += 512
                vt, voff = pqs[(QCOLS + KCOLS) // 512], (QCOLS + KCOLS) % 512
                _copy(nc, VCOPY_ENG, v_sb[:, lb, :],
                      vt[:, voff:voff + KCOLS])

                if lb % ROPE_GRP == 0:
                    state["rope_t"] = ropetab.tile(
                        [128, ROPE_GRP, 8, HH], BF16, tag="rope", bufs=2,
                        name=f"rope_{lb}")
                    nc.scalar.dma_start(
                        out=state["rope_t"], in_=rope_d[lb // ROPE_GRP],
                    )
                rope_t = state["rope_t"]
                rq = rope_t[:, lb % ROPE_GRP, 0:4, :]
                rk = rope_t[:, lb % ROPE_GRP, 4:8, :]

                qkn = work.tile([128, nqk * H], BF16, tag="qkn", bufs=5,
                                name=f"qkn_{lb}")
                ssq6 = stats.tile([128, 8], F32, tag="ssq6")
                rstd6 = stats.tile([128, 8], F32, tag="rstd6")
                for h in range(nqk):
                    head_stats(qkraw[:, h * H:(h + 1) * H], ssq6, h)
                finish_stats(ssq6, rstd6)
                for h in range(nqk):
                    rope_head(
                        qkraw[:, h * H:(h + 1) * H], rstd6[:, h:h + 1],
                        rq if h < nq else rk, qkn, h,
                    )
                depth = 3 if DELAY_T else 0
                state["pending"].append((qkn, lb))
                while len(state["pending"]) > depth:
                    transpose_block(*state["pending"].pop(0))

            def flush_pending():
                while state["pending"]:
                    transpose_block(*state["pending"].pop(0))

            # ---- phase D units: one (lb, db) output-projection tile ----
            n_db = D_ // 512
            dstate = {"ot": None, "ocount": 0}

            def emit_d_unit(it, engs=("vector", "scalar")):
                try:
                    lb, db = next(it)
                except StopIteration:
                    return False
                if db == 0:
                    ot_tile = outp.tile([128, D_], F32, tag="ot",
                                        name=f"ot_{lb}")
                    dstate["ot"] = ot_tile
                ot = dstate["ot"]
                po = psum.tile([128, 512], F32, tag="proj", bufs=3,
                               name=f"po_{lb}_{db}")
                for hh in range(nq):
                    nc.tensor.matmul(
                        po,
                        qkvT_sb[:, hh, lb * 128:(lb + 1) * 128],
                        wo_sb[:, hh, db * 512:(db + 1) * 512],
                        start=(hh == 0), stop=(hh == nq - 1),
                        skip_group_check=True,
                    )
                eng = (engs[dstate["ocount"] % len(engs)]
                       if OCOPY_SPLIT else "vector")
                dstate["ocount"] += 1
                oslab = ot[:, db * 512:(db + 1) * 512]
                if eng == "gpsimd":
                    nc.gpsimd.tensor_copy(out=oslab, in_=po)
                else:
                    _copy(nc, eng, oslab, po)
                if lb == n_lb - 1:
                    # split the very last row-block's DMA to shorten the tail
                    nc.sync.dma_start(
                        out=out_d[lb * 128:(lb + 1) * 128,
                                  db * 512:(db + 1) * 512],
                        in_=oslab,
                    )
                elif db == n_db - 1:
                    nc.sync.dma_start(
                        out=out_d[lb * 128:(lb + 1) * 128, :], in_=ot,
                    )
                return True

            def d_iter_for(gs):
                return iter([(lb, db)
                             for g in gs
                             for lb in range(4 * g, min(4 * g + 4, n_lb))
                             for db in range(n_db)])

            def drain_d(it, engs=("vector", "scalar")):
                while emit_d_unit(it, engs):
                    pass

            # ---- phase C: attention for one (head, 512-wide q group) ----
            cstate = {"fin_a": None, "fin_b": None}

            def run_fin_a():
                if cstate["fin_a"] is not None:
                    cstate["fin_a"]()
                    cstate["fin_a"] = None

            def run_fin_b():
                run_fin_a()
                if cstate["fin_b"] is not None:
                    cstate["fin_b"]()
                    cstate["fin_b"] = None

            def c_head(lqb, qh, d_iter=None, d_every=2,
                       d_engs=("vector", "scalar")):
                l0 = lqb * 512
                n_sc = min(n_lb, (l0 + 512) // 128) if causal else n_lb
                kv = qh // (nq // nkv)
                pav = psum.tile([128, 512], F32, tag="av", bufs=2,
                                name=f"av_{qh}_{lqb}")
                # running sum of exp rows (softmax denominator), built on
                # DVE so no PE ones-matmul per chunk is needed
                exsum = expp.tile([128, 512], BF16, tag="exsum", bufs=3,
                                  name=f"exsum_{qh}_{lqb}")
                if not causal:
                    mrows = maskp.tile([128, n_lb, 512], BF16, tag="mask")
                    nc.scalar.dma_start(
                        out=mrows,
                        in_=maskT_d[:, l0:l0 + 512].rearrange(
                            "(sb p) l -> p sb l", p=128
                        ),
                    )
                exps = []

                def av_mm(ex, sc, off):
                    # diagonal chunks only have live columns f >= s0-l0
                    nc.tensor.matmul(
                        pav[:, off:512],
                        v_sb[:, sc, kv * H:(kv + 1) * H], ex,
                        start=(sc == 0), stop=(sc == n_sc - 1),
                        skip_group_check=True,
                    )

                # software pipeline: av(c-DEPTH) after qk(c) so the
                # exp+mask latency of chunk c never stalls PE
                DEPTH = C_DEPTH if n_sc > C_DEPTH else max(1, n_sc - 1)

                def chunk_off(sc):
                    return max(0, sc * 128 - l0) if causal else 0

                for sc in range(n_sc):
                    off = chunk_off(sc)
                    w = 512 - off
                    ps = psum.tile([128, 512], F32, tag="scores",
                                   bufs=SCORES_BUFS,
                                   name=f"sc_{qh}_{lqb}_{sc}")
                    nc.tensor.matmul(
                        ps[:, 0:w],
                        qkT_sb[:, nq + kv, sc * 128:(sc + 1) * 128],
                        qkT_sb[:, qh, l0 + off:l0 + 512],
                        start=True, stop=True,
                    )
                    ex = expp.tile([128, 512], BF16, tag="exp")
                    nc.scalar.activation(
                        out=ex[:, 0:w], in_=ps[:, 0:w],
                        func=mybir.ActivationFunctionType.Exp, scale=SCALE,
                    )
                    if causal and sc * 128 > l0 - 128:
                        # keep where s0+p <= l0+off+f; base is always 0 here
                        # so only the leading 128 columns can be masked --
                        # beyond f=128 > p_max the predicate always holds.
                        # During the B-interleaved groups Pool is busy with
                        # k-rope, so mask there via the DVE triangle multiply
                        mw = min(w, 128 - (l0 + off - sc * 128))
                        if lqb <= 2:
                            nc.vector.tensor_mul(
                                ex[:, 0:mw], ex[:, 0:mw], tri_sb[:, 0:mw]
                            )
                        else:
                            nc.gpsimd.affine_select(
                                out=ex[:, 0:mw], in_=ex[:, 0:mw],
                                pattern=[[1, mw]],
                                compare_op=mybir.AluOpType.is_ge, fill=0.0,
                                base=l0 + off - sc * 128,
                                channel_multiplier=-1,
                            )
                    if not causal:
                        nc.vector.tensor_mul(ex, ex, mrows[:, sc, :])
                    # denominator accumulation on DVE; the first two full
                    # chunks fuse into a single 3-operand add
                    if sc == 0:
                        if n_sc == 1 or chunk_off(1) != 0:
                            nc.vector.tensor_copy(out=exsum, in_=ex)
                    elif sc == 1 and off == 0:
                        nc.vector.tensor_add(exsum, exps[0][0], ex)
                    else:
                        nc.vector.tensor_add(
                            exsum[:, off:512], exsum[:, off:512],
                            ex[:, 0:w],
                        )
                    exps.append((ex[:, 0:w], sc, off))
                    if sc >= DEPTH:
                        av_mm(*exps[sc - DEPTH])
                    if sc == 1:
                        run_fin_a()  # previous head's den sum + reciprocal
                    elif sc == max(2, min(4, n_sc - 1)):
                        run_fin_b()  # previous head's broadcast + normalize
                    if d_iter is not None and sc % d_every == d_every - 1:
                        emit_d_unit(d_iter, d_engs)
                for sc in range(max(0, n_sc - DEPTH), n_sc):
                    av_mm(*exps[sc])
                if n_sc <= 1:
                    run_fin_b()

                # denominator: one ones-matmul over the accumulated exsum
                # (vs one per chunk), reciprocal, then a PE partition
                # broadcast in f32r and the final normalize on DVE.
                # Deferred in two stages into the next head's chunk stream so
                # neither the scores-ring slots nor the DVE chain stall PE.
                fstate = {}

                def fin_a():
                    if DEN_MODE == "pool":
                        den = stats.tile([1, 512], F32, tag="den", bufs=2)
                        nc.gpsimd.tensor_reduce(
                            den, exsum, mybir.AxisListType.C,
                            mybir.AluOpType.add,
                        )
                    else:
                        den = psum.tile([1, 512], F32, tag="proj",
                                        bufs=3,
                                        name=f"pden_{qh}_{lqb}")
                        nc.tensor.matmul(den, ones_sb, exsum,
                                         start=True, stop=True)
                    rden = stats.tile([1, 512], F32R, tag="rden", bufs=2)
                    with nc.allow_low_precision(
                        reason="f32r reciprocal keeps full fp32 bits"
                    ):
                        nc.vector.reciprocal(out=rden, in_=den)
                    fstate["rden"] = rden

                def fin_b():
                    pbc = psum.tile([128, 512], F32, tag="scores",
                                    bufs=SCORES_BUFS, name=f"bc_{qh}_{lqb}")
                    nc.tensor.matmul(pbc, onesf_sb, fstate["rden"],
                                     start=True, stop=True)
                    rdenb = work.tile([128, 512], F32, tag="rdenb", bufs=2)
                    nc.vector.tensor_copy(out=rdenb, in_=pbc)
                    nc.vector.tensor_mul(
                        qkvT_sb[:, qh, l0:l0 + 512], pav, rdenb
                    )

                cstate["fin_a"] = fin_a
                cstate["fin_b"] = fin_b

            def c_block(lqb, d_iter=None, d_every=2,
                        d_engs=("vector", "scalar"), last_engs=None):
                for qh in range(nq):
                    eng = (last_engs if (last_engs and qh == nq - 1)
                           else d_engs)
                    c_head(lqb, qh, d_iter, d_every, eng)

            # ---- last output-projection group, hh-outer so only the final
            # 4 matmuls per wave wait on the last head's qkvT write ----
            d3_lbs = list(range(4 * (n_lqb - 1), n_lb))
            D3_TAGS = [("proj", 3), ("proj", 3), ("proj", 3),
                       ("scores", SCORES_BUFS)]

            def d3_wave_alloc(db):
                pos = {}
                for i, lb in enumerate(d3_lbs):
                    tg, bf = D3_TAGS[i % len(D3_TAGS)]
                    pos[lb] = psum.tile([128, 512], F32, tag=tg, bufs=bf,
                                        name=f"po3_{lb}_{db}")
                return pos

            def d3_mm(pos, db, hh_range):
                for hh in hh_range:
                    for lb in d3_lbs:
                        nc.tensor.matmul(
                            pos[lb],
                            qkvT_sb[:, hh, lb * 128:(lb + 1) * 128],
                            wo_sb[:, hh, db * 512:(db + 1) * 512],
                            start=(hh == 0), stop=(hh == nq - 1),
                            skip_group_check=True,
                        )

            def d3_finish_wave(pos, db):
                for i, lb in enumerate(d3_lbs):
                    oslab = outp.tile([128, 512], F32, tag="ot3", bufs=6,
                                      name=f"ot3_{lb}_{db}")
                    _copy(nc, ("vector", "scalar")[i % 2], oslab, pos[lb])
                    nc.sync.dma_start(
                        out=out_d[lb * 128:(lb + 1) * 128,
                                  db * 512:(db + 1) * 512],
                        in_=oslab,
                    )

            def d3_fused(pre=None):
                for db in range(n_db):
                    if pre is not None and db == 0:
                        pos = pre
                        d3_mm(pos, db, range(nq - 1, nq))
                    else:
                        pos = d3_wave_alloc(db)
                        d3_mm(pos, db, range(nq))
                    d3_finish_wave(pos, db)

            # ---- schedule ----
            if INTERLEAVE == "fine" and causal and n_lqb == 4 and n_lb == 16:
                xts = preload_startup()
                b_fused_start({k: v for k, v in xts.items() if k < 3})
                for lb in range(3, 9):
                    b_block(lb, xts.get(lb))
                if WO_LATE:
                    load_wo()
                del xts
                # C0/C1 head-units between the remaining B blocks: PE keeps
                # crunching proj matmuls while ACT chews the C exps.  The
                # last C1 heads (issued after every B block) already pull in
                # D0 units to plug PE bubbles from the exp backlog.
                it01 = d_iter_for([0, 1])
                cunits = ([(0, q, None) for q in range(nq)]
                          + [(1, 0, None), (1, 1, None),
                             (1, 2, it01), (1, 3, it01)])
                rest_b = list(range(9, n_lb))
                while cunits or rest_b:
                    if rest_b:
                        b_block(rest_b.pop(0))
                    if cunits:
                        lqb, qh, dit = cunits.pop(0)
                        c_head(lqb, qh, dit, d_every=2)
                # transposes for lb 13..15 stay pending: C2 needs only lb<=11;
                # pop them one per C2 head so their copies spread out
                for qh in range(nq):
                    c_head(2, qh, it01, d_every=2)
                    if state["pending"]:
                        transpose_block(*state["pending"].pop(0))
                flush_pending()
                drain_d(it01)
                it2 = d_iter_for([2])
                c_block(3, it2, d_every=4, d_engs=("vector",))
                # wave-0 partial sums for heads 0..2 keep PE busy while the
                # last head's denominator chain drains
                pre = d3_wave_alloc(0)
                d3_mm(pre, 0, range(nq - 1))
                run_fin_b()
                drain_d(it2, engs=("vector", "scalar"))
                d3_fused(pre)
            else:
                xts = preload_startup()
                for lb in range(n_lb):
                    b_block(lb, xts.get(lb))
                if WO_LATE:
                    load_wo()
                for g in range(n_lqb):
                    if g == n_lqb - 1 or not causal:
                        flush_pending()
                    c_block(g)
                    flush_pending()
                run_fin_b()
                drain_d(d_iter_for(list(range(n_lqb))))
    return nc


# ---------------- host side ----------------

def _x_block(xb, L_=L, D_=D):
    """Host-preblocked x: [lb, p, dc*128] with element x[lb*128+l, dc*128+p]."""
    n_lb, n_dc = L_ // 128, D_ // 128
    y = xb.reshape(n_lb, 128, n_dc, 128).transpose(0, 3, 2, 1)
    return np.ascontiguousarray(y).reshape(n_lb, 128, D_)


def _rope_block(pos, qw, kw, L_=L):
    """Host-preblocked rope tables: [n_grp, 128, grp*8*(H//2)] bf16 where
    element [gi, p, (Bi, a, j)] = table[a, (gi*grp+Bi)*128 + p, j]."""
    tabs = np.concatenate([_rope_tables(pos, qw), _rope_tables(pos, kw)])
    n_lb = L_ // 128
    grp = 4 if n_lb % 4 == 0 else 1
    t = tabs.reshape(8, n_lb // grp, grp, 128, H // 2)
    return np.ascontiguousarray(t.transpose(1, 3, 2, 0, 4)).reshape(
        n_lb // grp, 128, grp * 8 * (H // 2)
    ).astype(ml_dtypes.bfloat16)


def _rope_tables(pos, norm_w):
    """A,B,C,D [4, L, H/2] f32 with the rms-norm weight folded in.
    h1 = q1*A - q2*B ; h2 = q2*C + q1*D  (q already divided by rms)."""
    hh = H // 2
    fraction = 2.0 * np.arange(hh, dtype=np.float32) / np.float32(H)
    timescale = np.float32(ROPE_THETA) ** fraction
    sinusoid = pos.astype(np.float32)[:, None] / timescale[None, :]
    sin = np.sin(sinusoid).astype(np.float32)
    cos = np.cos(sinusoid).astype(np.float32)
    w1 = norm_w[:hh].astype(np.float32)
    w2 = norm_w[hh:].astype(np.float32)
    return np.stack([cos * w1, sin * w2, cos * w2, sin * w1]).astype(np.float32)


_KERNELS = {}
TRACE = False
LAST_RESULTS = None


def _get_kernel(causal):
    if causal not in _KERNELS:
        _KERNELS[causal] = build_core_kernel(causal=causal)
    return _KERNELS[causal]


def kernel(**inputs):
    x = np.asarray(inputs["x"], dtype=np.float32)
    pos = np.asarray(inputs["position_ids"])
    mask = np.asarray(inputs["attn_mask"]).astype(bool)
    wq = np.asarray(inputs["wq"], dtype=np.float32)
    wk = np.asarray(inputs["wk"], dtype=np.float32)
    wv = np.asarray(inputs["wv"], dtype=np.float32)
    wo = np.asarray(inputs["wo"], dtype=np.float32)
    qw = np.asarray(inputs["q_norm_w"], dtype=np.float32)
    kw = np.asarray(inputs["k_norm_w"], dtype=np.float32)

    tril = np.tril(np.ones((L, L), dtype=bool))
    causal = all(np.array_equal(mask[b], tril) for b in range(B))
    nc = _get_kernel(causal)

    bf = ml_dtypes.bfloat16
    per_batch = []
    for b in range(B):
        d = {
            "xT": _x_block(x[b].astype(bf)),
            "rope": _rope_block(pos[b], qw, kw),
        }
        if not causal:
            d["maskT"] = np.ascontiguousarray(mask[b].T).astype(bf)
        per_batch.append(d)

    in_maps = []
    for c in range(N_CORES):
        b, g = divmod(c, N_CORES // B)
        qs = slice(QH_PER_CORE * g, QH_PER_CORE * (g + 1))
        ks = slice(KV_PER_CORE * g, KV_PER_CORE * (g + 1))
        wqkv = np.concatenate(
            [
                wq[:, qs, :].reshape(D, QH_PER_CORE * H),
                wk[:, ks, :].reshape(D, KV_PER_CORE * H),
                wv[:, ks, :].reshape(D, KV_PER_CORE * H),
            ],
            axis=1,
        ).astype(bf)
        m = dict(per_batch[b])
        m["wqkv"] = wqkv
        m["wo"] = np.ascontiguousarray(wo[qs].reshape(QH_PER_CORE * H, D)).astype(bf)
        in_maps.append(m)

    global LAST_RESULTS
    res = run_bass_kernel_spmd(
        nc, in_maps, core_ids=list(range(N_CORES)), trace=TRACE
    )
    LAST_RESULTS = res
    out = np.zeros((B, L, D), dtype=np.float32)
    for c in range(N_CORES):
        out[c // (N_CORES // B)] += res.results[c]["out"]
    return out



# revision 115
# speedup vs baseline: 1.0024x; 1.0024x over previous
"""Trainium2 Bass kernel for a GQA attention block (B=2, L=2048, D=2048,
16 q-heads / 8 kv-heads, head_dim=128), sharded over 8 NeuronCores.

Sharding: core c -> batch b = c // 4, head-group g = c % 4 (4 q-heads and
their 2 kv-heads).  Each core computes its heads' attention plus the partial
output projection; the host sums the 4 partials per batch.

Self-contained: only needs numpy / ml_dtypes / concourse (on PYTHONPATH in
this container).
"""

import math
import sys

for _p in ("/root/.axon_site", "/root/.axon_site/_ro/trn_rl_repo",
           "/root/.axon_site/_ro/pypackages"):
    if _p not in sys.path:
        sys.path.append(_p)

import numpy as np
import ml_dtypes

import concourse.bass as bass
import concourse.bass2jax as bass2jax
import concourse.bass_isa as bass_isa
import concourse.mybir as mybir
import concourse.tile as tile
from concourse.masks import make_identity
from concourse.bass_utils import run_bass_kernel_spmd
from concourse.vector_clock import ScopedClock, VectorClock


def _legalize_bir_waits(bir_bytes):
    """This walrus build supports only ONE sync-wait slot per instruction.
    Hoist extra waits onto NoOp instructions inserted just before the
    offender (same engine, so the engine stream still blocks in order)."""
    import orjson

    d = orjson.loads(bir_bytes)
    n_split = 0
    for f in d["functions"]:
        for bb in f["blocks"]:
            out = []
            for inst in bb["instructions"]:
                si = inst.get("sync_info")
                waits = (si or {}).get("on_wait") or []
                if len(waits) > 1:
                    for j, w in enumerate(waits[:-1]):
                        n_split += 1
                        out.append({
                            "engine": inst["engine"], "ins": [], "outs": [],
                            "name": f"{inst['name']}__w{j}",
                            "opcode": "NoOp",
                            "sync_info": {"on_wait": [w], "on_update": []},
                        })
                    si["on_wait"] = [waits[-1]]
                out.append(inst)
            bb["instructions"] = out
    return orjson.dumps(d)


_orig_compile_bir_kernel = bass2jax.compile_bir_kernel


def _patched_compile_bir_kernel(ant_bir_str, *args, **kwargs):
    return _orig_compile_bir_kernel(_legalize_bir_waits(ant_bir_str), *args, **kwargs)


bass2jax.compile_bir_kernel = _patched_compile_bir_kernel

BF16 = mybir.dt.bfloat16
F32 = mybir.dt.float32

# Full-problem constants
B, L, D = 2, 2048, 2048
N_HEADS, N_KV, H = 16, 8, 128
EPS = 1e-6
ROPE_THETA = 1e6
N_CORES = 8
QH_PER_CORE = N_HEADS // (N_CORES // B)   # 4
KV_PER_CORE = N_KV // (N_CORES // B)      # 2
SCALE = H ** -0.5


class PatchedTileContext(tile.TileContext):
    """This walrus build only supports one sync-wait slot on a CTRL (Drain)
    instruction; split the tail-drain waits across one drain per processor."""

    def _drain_and_barrier(self, tick_clock, wait_clock):
        gc = tick_clock.global_clock
        n = len(gc)
        for p in range(n):
            t = gc[p]
            if t > 0:
                vc = VectorClock([t if i == p else 0 for i in range(n)])
                d = self.nc.sync.drain()
                wait_clock.add_sem_waits(d.ins, ScopedClock({None: vc}))
                si = d.ins.sync_info
                nw = len(si.on_wait) if si is not None else 0
                assert nw <= 1, f"proc {p} produced {nw} waits"
        self.nc.all_engine_barrier()
        assert self.sems is not None
        popped = self.nc._tile_sem_poison_stack.pop()
        assert popped is self._sem_poison
        self.nc.clear_and_free_semaphores(list(self.sems.allocated().values()))
        self.nc.all_engine_barrier()


# engine assignment knobs (tuned against the cost-model timeline)
TCOPY_ENG = "scalar"   # transpose PSUM->SBUF copies (gpsimd cannot touch PSUM)
VCOPY_ENG = "vector"  # v PSUM->SBUF copy
OCOPY_SPLIT = True     # split phase-D copies between DVE and ACT
DELAY_T = True         # transpose qkn one L-block late
ROPE_STT = True        # fold rstd into stt ops reading the SBUF staging
WQKV_SPLIT = True      # split the wqkv load into 4 chunks after xt(0)
WO_LATE = True         # load wo just before it's needed
INTERLEAVE = "fine"    # "fine": B/C head interleave + C/D unit interleave
PE_WARMUP = 0
QKRAW_ENG = "scalar"   # staging copies of q+k PSUM->SBUF
C_DEPTH = 5            # attention chunk software-pipeline depth
EXP_BUFS = 7
ROPE_K_ENG = "gpsimd"  # engine for k-head rope multiplies
WQKV_FIRST1 = False    # make the first wqkv chunk a single dc
STATS_BUFS = 6
WORK_BUFS = 3
SCORES_BUFS = 3        # scores PSUM ring (av 3 + scores 3 + proj 2 = 8 banks)
DEN_MODE = "pool"      # "pool": gpsimd C-axis reduce; "mm": PE ones-matmul


def _copy(nc, eng, out, in_):
    if eng == "vector":
        nc.vector.tensor_copy(out=out, in_=in_)
    elif eng == "gpsimd":
        nc.gpsimd.tensor_copy(out=out, in_=in_)
    else:
        nc.scalar.copy(out=out, in_=in_)


def build_core_kernel(L_=L, D_=D, nq=QH_PER_CORE, nkv=KV_PER_CORE, causal=True):
    """One core's program.  Inputs (DRAM):
      xT    [L/128, 128, D] bf16 — host-preblocked x (see _x_block)
      wqkv  [D, nq*H + 2*nkv*H] bf16  ([wq heads | wk heads | wv heads])
      wo    [nq*H, D] bf16
      rope  [L/128/G, 128, G*8*(H/2)] f32 — preblocked A,B,C,D cos/sin
            tables for q then k, norm weights folded in (see _rope_block)
      maskT [L, L] bf16 (only if causal=False; 0/1 multiplicative, [s, l])
    Output:
      out [L, D] bf16 — partial sum over this core's heads (the host
      accumulates the four per-batch partials in f32).

    Layout strategy: projections produce q/k/v in natural [L-part, H] layout
    (RMS-norm + RoPE are row-wise there, all bf16 so DVE gets its 2x/4x
    modes; k-head rope multiplies run on gpsimd), q/k are transposed per
    128-block on the PE (identity matmul); attention runs fully transposed —
    scores^T = kT^T qT, exp on ACT, causal mask via a 128-col affine_select
    (gpsimd) or triangle multiply (DVE), av^T accumulated over S-chunks in
    PSUM — so av^T feeds the output projection as lhsT with zero further
    transposes.  The softmax denominator is a bf16 running sum of the exp
    tiles on DVE, reduced across partitions once per (head, q-group) by a
    gpsimd C-axis tensor_reduce, then broadcast by a tiny f32r PE matmul;
    the whole finalize is deferred into the next head's chunk stream.
    Schedule: dc-interleaved fused start for blocks 0-2 chasing the
    dc-ordered wqkv/x DMA queue, C0/C1 head-units between the later B
    blocks, output-projection (lb,db)-units woven into C2/C3's chunk
    streams, and an hh-outer fused final D group.
    """
    HH = H // 2
    n_lb = L_ // 128          # L blocks of 128
    n_dc = D_ // 128          # D contraction chunks
    n_lqb = L_ // 512         # q blocks of 512
    QCOLS = nq * H
    KCOLS = nkv * H
    KV_COLS = 2 * nkv * H
    W_COLS = QCOLS + KV_COLS
    assert W_COLS % 512 == 0
    n_wslab = W_COLS // 512   # 512-wide slabs of the qkv projection

    nc = bass.Bass()
    # x, host-preblocked: [lb, p, dc*128] with element = x[lb*128+l, dc*128+p]
    xT_d = nc.dram_tensor("xT", [L_ // 128, 128, D_], BF16, kind="ExternalInput")
    wqkv_d = nc.dram_tensor("wqkv", [D_, W_COLS], BF16, kind="ExternalInput")
    wo_d = nc.dram_tensor("wo", [QCOLS, D_], BF16, kind="ExternalInput")
    # rope tables, host-preblocked: [group, p, (lb-in-group, table 0..7, j)]
    ROPE_GRP = 4 if (L_ // 128) % 4 == 0 else 1
    rope_d = nc.dram_tensor(
        "rope", [L_ // 128 // ROPE_GRP, 128, ROPE_GRP * 8 * HH], BF16,
        kind="ExternalInput",
    )
    if not causal:
        maskT_d = nc.dram_tensor("maskT", [L_, L_], BF16, kind="ExternalInput")
    out_d = nc.dram_tensor("out", [L_, D_], BF16, kind="ExternalOutput")
    nqk = nq + nkv  # q heads then k heads in the combined transposed tile

    with PatchedTileContext(nc) as tc:
        with (
            tc.tile_pool(name="res", bufs=1) as res,
            tc.tile_pool(name="ropetab", bufs=3) as ropetab,
            tc.tile_pool(name="work", bufs=WORK_BUFS) as work,
            tc.tile_pool(name="stats", bufs=STATS_BUFS) as stats,
            tc.tile_pool(name="expp", bufs=EXP_BUFS) as expp,
            tc.tile_pool(name="outp", bufs=3) as outp,
            tc.tile_pool(name="psum", bufs=1, space="PSUM") as psum,
            tc.tile_pool(name="maskp", bufs=2) as maskp,
        ):
            # ---- resident loads ----
            wqkv_sb = res.tile([128, n_dc, W_COLS], BF16, tag="wqkv")
            wqkv_r = wqkv_d.rearrange("(dc p) c -> p dc c", p=128)
            wo_sb = res.tile([128, nq, D_], BF16, tag="wo")

            def load_wo():
                nc.scalar.dma_start(
                    out=wo_sb, in_=wo_d.rearrange("(hh p) d -> p hh d", p=128)
                )

            if not WO_LATE:
                load_wo()
            ones_sb = res.tile([128, 1], BF16, tag="ones")
            nc.vector.memset(ones_sb, 1.0)
            # f32r ones row: the partition-broadcast matmul runs at bf16
            # speed for N>=256 while keeping full fp32 mantissa bits
            F32R = mybir.dt.float32r
            onesf_sb = res.tile([1, 128], F32R, tag="onesf")
            onesf_f32 = res.tile([1, 128], F32, tag="onesf32")
            nc.vector.memset(onesf_f32, 1.0)
            with nc.allow_low_precision(reason="f32r ones"):
                nc.vector.tensor_copy(out=onesf_sb, in_=onesf_f32)
            eps_sb = res.tile([128, 1], F32, tag="eps")
            nc.vector.memset(eps_sb, EPS)
            ident_sb = res.tile([128, 128], BF16, tag="ident")
            make_identity(nc, ident_sb)
            # 0/1 lower-triangle (keep f >= p) for the causal mask multiply
            tri_sb = res.tile([128, 128], BF16, tag="tri")
            nc.vector.memset(tri_sb, 1.0)
            nc.gpsimd.affine_select(
                out=tri_sb, in_=tri_sb, pattern=[[1, 128]],
                compare_op=mybir.AluOpType.is_ge, fill=0.0,
                base=0, channel_multiplier=-1,
            )
            if PE_WARMUP:
                pw = psum.tile([128, 128], BF16, tag="scores", bufs=3,
                               name="pe_warm")
                for _ in range(PE_WARMUP):
                    nc.tensor.transpose(pw, ident_sb, ident_sb)

            v_sb = res.tile([128, n_lb, KCOLS], BF16, tag="v")
            qkT_sb = res.tile([128, nqk, L_], BF16, tag="qkT")
            qkvT_sb = res.tile([128, nq, L_], BF16, tag="qkvT")

            # ---- phase B: qkv projection + rmsnorm + rope + transposes ----
            def head_stats(src, ssq6, h, eng="vector"):
                """sum(x^2) for one head: stt square with accumulator
                (x*1)*x; DVE gets it cheap in bf16, but the fused-start
                blocks use ACT Square to spare DVE's post-fused burst."""
                sq = work.tile([128, H], BF16, tag="sq")
                if eng == "act":
                    nc.scalar.activation(
                        out=sq, in_=src,
                        func=mybir.ActivationFunctionType.Square,
                        accum_out=ssq6[:, h:h + 1],
                    )
                else:
                    nc.vector.scalar_tensor_tensor(
                        out=sq, in0=src, scalar=1.0, in1=src,
                        op0=mybir.AluOpType.mult, op1=mybir.AluOpType.mult,
                        accum_out=ssq6[:, h:h + 1],
                    )

            def finish_stats(ssq6, rstd6):
                # one fused sqrt / reciprocal for all heads of the block
                nc.scalar.activation(
                    out=rstd6[:, 0:nqk], in_=ssq6[:, 0:nqk],
                    func=mybir.ActivationFunctionType.Sqrt,
                    bias=eps_sb, scale=1.0 / H,
                )
                nc.vector.reciprocal(out=rstd6[:, 0:nqk], in_=rstd6[:, 0:nqk])

            def rope_head(src, rstd, rtab, qkn, dcol):
                """RoPE one head (src: [128, H] bf16 in SBUF); bf16 for DVE
                2x/4x perf modes:
                h1 = (q1*rstd)*A - (q2*rstd)*B ; h2 = (q2*rstd)*C + (q1*rstd)*D
                k heads run their multiplies on gpsimd (plain tensor_tensor
                only -- Pool has no stt opcode), with the rstd prescale done
                on DVE where tensor_scalar gets the 4x mode."""
                qb = qkn[:, dcol * H:(dcol + 1) * H]
                mul = mybir.AluOpType.mult
                on_pool = dcol >= nq and ROPE_K_ENG == "gpsimd"
                veng = nc.gpsimd if on_pool else nc.vector
                if on_pool:
                    qn = work.tile([128, H], BF16, tag="qn")
                    nc.vector.tensor_scalar_mul(qn, src, rstd)
                    s1, s2 = qn[:, 0:HH], qn[:, HH:H]
                else:
                    s1, s2 = src[:, 0:HH], src[:, HH:H]
                # separate tile rings per engine: sharing them would make
                # DVE waits ride on Pool's slower ops via slot reuse
                tga, tgb = ("kt1", "kt2") if on_pool else ("t1", "t2")
                t1 = work.tile([128, HH], BF16, tag=tga, name="t1")
                t2 = work.tile([128, HH], BF16, tag=tgb, name="t2")

                def rmul(out, sx, tab):
                    if on_pool:
                        veng.tensor_mul(out, sx, tab)
                    else:
                        veng.scalar_tensor_tensor(
                            out=out, in0=sx, scalar=rstd, in1=tab,
                            op0=mul, op1=mul)

                rmul(t1, s1, rtab[:, 0, :])
                rmul(t2, s2, rtab[:, 1, :])
                veng.tensor_sub(qb[:, 0:HH], t1, t2)
                t3 = work.tile([128, HH], BF16, tag=tga, name="t3")
                t4 = work.tile([128, HH], BF16, tag=tgb, name="t4")
                rmul(t3, s2, rtab[:, 2, :])
                rmul(t4, s1, rtab[:, 3, :])
                veng.tensor_add(qb[:, HH:H], t3, t4)

            def transpose_block(qkn, lb):
                # transpose each head block on PE (identity matmul)
                for h in range(nqk):
                    pt = psum.tile([128, 128], BF16, tag="scores", bufs=SCORES_BUFS,
                                   name=f"pt_{lb}_{h}")
                    nc.tensor.transpose(
                        pt, qkn[:, h * H:(h + 1) * H], ident_sb
                    )
                    _copy(nc, TCOPY_ENG,
                          qkT_sb[:, h, lb * 128:(lb + 1) * 128], pt)

            state = {"pending": []}  # [(qkn, lb)] transposed T_DELAY late

            def preload_startup():
                """Interleave the wqkv chunks with xt pieces for blocks 0/1
                in dc order on ONE queue, so early proj matmuls are gated by
                the minimum prefix of bytes rather than whole-tensor DMAs."""
                if not (WQKV_SPLIT and n_dc >= 8):
                    nc.scalar.dma_start(out=wqkv_sb, in_=wqkv_r)
                    return {}
                xts = {}
                for i in range(min(4, n_lb)):
                    t = work.tile([128, n_dc, 128], BF16, tag="xt", bufs=4,
                                  name=f"xt_{i}")
                    xts[i] = t
                # weight chunks in dc order; xt pieces (>=2 dc so each DMA
                # descriptor stays >=512B) slotted between them
                # block 0's first piece goes down the SP queue so its DMA
                # latency chain overlaps the first weight chunk's
                nc.sync.dma_start(out=xts[0][:, 0:2, :],
                                  in_=xT_d[0, :, 0:256])
                wb = [0, 1, 2, 3, 4, 6, 8, 10, 12, 14, n_dc]
                xb = [0, 2, 4, 6, 8, 10, 12, 14, n_dc]
                xi = 0
                for i, j in zip(wb[:-1], wb[1:]):
                    nc.scalar.dma_start(
                        out=wqkv_sb[:, i:j, :], in_=wqkv_r[:, i:j, :]
                    )
                    while xi + 1 < len(xb) and xb[xi + 1] <= j:
                        a, b_ = xb[xi], xb[xi + 1]
                        for bi, t in xts.items():
                            if bi == 3 or (bi == 0 and a == 0):
                                continue  # block 3 loads after the hot path
                            nc.scalar.dma_start(
                                out=t[:, a:b_, :],
                                in_=xT_d[bi, :, a * 128:b_ * 128],
                            )
                        xi += 1
                if 3 in xts:
                    nc.scalar.dma_start(out=xts[3], in_=xT_d[3])
                return xts

            def b_fused_start(xts):
                """Projection matmuls for blocks 0..2 interleaved by dc so
                PE consumption tracks the dc-ordered DMA arrival; one PSUM
                accumulator pair per block drawn from the three tag rings."""
                tags = [("proj", 3), ("scores", SCORES_BUFS), ("av", 2)]
                pqs = {}
                for bi in xts:
                    tg, bf = tags[bi]
                    pqs[bi] = [psum.tile([128, 512], F32, tag=tg, bufs=bf,
                                         name=f"projf_{bi}_{s}")
                               for s in range(n_wslab)]
                for dc in range(n_dc):
                    for bi, t in xts.items():
                        for s in range(n_wslab):
                            nc.tensor.matmul(
                                pqs[bi][s],
                                t[:, dc, :],
                                wqkv_sb[:, dc, s * 512:(s + 1) * 512],
                                start=(dc == 0), stop=(dc == n_dc - 1),
                                skip_group_check=True,
                            )
                for bi in xts:
                    b_post(bi, pqs[bi])

            def b_block(lb, xt_pre=None):
                if xt_pre is not None:
                    xt = xt_pre
                else:
                    xt = work.tile([128, n_dc, 128], BF16, tag="xt", bufs=4,
                                   name=f"xt_{lb}")
                    nc.sync.dma_start(out=xt, in_=xT_d[lb])
                # dc-outer: both slabs accumulate in parallel, so early
                # weight chunks enable matmuls in arrival order
                pqs = [psum.tile([128, 512], F32, tag="proj", bufs=3,
                                 name=f"proj_{lb}_{s}")
                       for s in range(n_wslab)]
                for dc in range(n_dc):
                    for s in range(n_wslab):
                        nc.tensor.matmul(
                            pqs[s],
                            xt[:, dc, :],
                            wqkv_sb[:, dc, s * 512:(s + 1) * 512],
                            start=(dc == 0), stop=(dc == n_dc - 1),
                            skip_group_check=True,
                        )
                b_post(lb, pqs)

            def b_post(lb, pqs):
                # bulk-stage q+k to SBUF (bf16 so rope DVE ops get 2x/4x);
                # v goes straight to its resident tile
                qkraw = work.tile([128, QCOLS + KCOLS], BF16, tag="qkraw",
                                  bufs=3, name=f"qkraw_{lb}")
                off = 0
                for s in range(n_wslab):
                    w = min(512, QCOLS + KCOLS - off)
                    if w > 0:
                        _copy(nc, QKRAW_ENG,
                              qkraw[:, off:off + w], pqs[s][:, 0:w])
                    off += 512
                vt, voff = pqs[(QCOLS + KCOLS) // 512], (QCOLS + KCOLS) % 512
                _copy(nc, VCOPY_ENG, v_sb[:, lb, :],
                      vt[:, voff:voff + KCOLS])

                if lb % ROPE_GRP == 0:
                    state["rope_t"] = ropetab.tile(
                        [128, ROPE_GRP, 8, HH], BF16, tag="rope", bufs=2,
                        name=f"rope_{lb}")
                    nc.scalar.dma_start(
                        out=state["rope_t"], in_=rope_d[lb // ROPE_GRP],
                    )
                rope_t = state["rope_t"]
                rq = rope_t[:, lb % ROPE_GRP, 0:4, :]
                rk = rope_t[:, lb % ROPE_GRP, 4:8, :]

                qkn = work.tile([128, nqk * H], BF16, tag="qkn", bufs=5,
                                name=f"qkn_{lb}")
                ssq6 = stats.tile([128, 8], F32, tag="ssq6")
                rstd6 = stats.tile([128, 8], F32, tag="rstd6")
                for h in range(nqk):
                    head_stats(qkraw[:, h * H:(h + 1) * H], ssq6, h)
                finish_stats(ssq6, rstd6)
                for h in range(nqk):
                    rope_head(
                        qkraw[:, h * H:(h + 1) * H], rstd6[:, h:h + 1],
                        rq if h < nq else rk, qkn, h,
                    )
                depth = 3 if DELAY_T else 0
                state["pending"].append((qkn, lb))
                while len(state["pending"]) > depth:
                    transpose_block(*state["pending"].pop(0))

            def flush_pending():
                while state["pending"]:
                    transpose_block(*state["pending"].pop(0))

            # ---- phase D units: one (lb, db) output-projection tile ----
            n_db = D_ // 512
            dstate = {"ot": None, "ocount": 0}

            def emit_d_unit(it, engs=("vector", "scalar")):
                try:
                    lb, db = next(it)
                except StopIteration:
                    return False
                if db == 0:
                    ot_tile = outp.tile([128, D_], BF16, tag="ot",
                                        name=f"ot_{lb}")
                    dstate["ot"] = ot_tile
                ot = dstate["ot"]
                po = psum.tile([128, 512], F32, tag="proj", bufs=3,
                               name=f"po_{lb}_{db}")
                for hh in range(nq):
                    nc.tensor.matmul(
                        po,
                        qkvT_sb[:, hh, lb * 128:(lb + 1) * 128],
                        wo_sb[:, hh, db * 512:(db + 1) * 512],
                        start=(hh == 0), stop=(hh == nq - 1),
                        skip_group_check=True,
                    )
                eng = (engs[dstate["ocount"] % len(engs)]
                       if OCOPY_SPLIT else "vector")
                dstate["ocount"] += 1
                oslab = ot[:, db * 512:(db + 1) * 512]
                if eng == "gpsimd":
                    nc.gpsimd.tensor_copy(out=oslab, in_=po)
                else:
                    _copy(nc, eng, oslab, po)
                if lb == n_lb - 1:
                    # split the very last row-block's DMA to shorten the tail
                    nc.sync.dma_start(
                        out=out_d[lb * 128:(lb + 1) * 128,
                                  db * 512:(db + 1) * 512],
                        in_=oslab,
                    )
                elif db == n_db - 1:
                    nc.sync.dma_start(
                        out=out_d[lb * 128:(lb + 1) * 128, :], in_=ot,
                    )
                return True

            def d_iter_for(gs):
                return iter([(lb, db)
                             for g in gs
                             for lb in range(4 * g, min(4 * g + 4, n_lb))
                             for db in range(n_db)])

            def drain_d(it, engs=("vector", "scalar")):
                while emit_d_unit(it, engs):
                    pass

            # ---- phase C: attention for one (head, 512-wide q group) ----
            cstate = {"fin_a": None, "fin_b": None}

            def run_fin_a():
                if cstate["fin_a"] is not None:
                    cstate["fin_a"]()
                    cstate["fin_a"] = None

            def run_fin_b():
                run_fin_a()
                if cstate["fin_b"] is not None:
                    cstate["fin_b"]()
                    cstate["fin_b"] = None

            def c_head(lqb, qh, d_iter=None, d_every=2,
                       d_engs=("vector", "scalar")):
                l0 = lqb * 512
                n_sc = min(n_lb, (l0 + 512) // 128) if causal else n_lb
                kv = qh // (nq // nkv)
                pav = psum.tile([128, 512], F32, tag="av", bufs=2,
                                name=f"av_{qh}_{lqb}")
                # running sum of exp rows (softmax denominator), built on
                # DVE so no PE ones-matmul per chunk is needed
                exsum = expp.tile([128, 512], BF16, tag="exsum", bufs=3,
                                  name=f"exsum_{qh}_{lqb}")
                if not causal:
                    mrows = maskp.tile([128, n_lb, 512], BF16, tag="mask")
                    nc.scalar.dma_start(
                        out=mrows,
                        in_=maskT_d[:, l0:l0 + 512].rearrange(
                            "(sb p) l -> p sb l", p=128
                        ),
                    )
                exps = []

                def av_mm(ex, sc, off):
                    # diagonal chunks only have live columns f >= s0-l0
                    nc.tensor.matmul(
                        pav[:, off:512],
                        v_sb[:, sc, kv * H:(kv + 1) * H], ex,
                        start=(sc == 0), stop=(sc == n_sc - 1),
                        skip_group_check=True,
                    )

                # software pipeline: av(c-DEPTH) after qk(c) so the
                # exp+mask latency of chunk c never stalls PE
                DEPTH = C_DEPTH if n_sc > C_DEPTH else max(1, n_sc - 1)

                def chunk_off(sc):
                    return max(0, sc * 128 - l0) if causal else 0

                for sc in range(n_sc):
                    off = chunk_off(sc)
                    w = 512 - off
                    ps = psum.tile([128, 512], F32, tag="scores",
                                   bufs=SCORES_BUFS,
                                   name=f"sc_{qh}_{lqb}_{sc}")
                    nc.tensor.matmul(
                        ps[:, 0:w],
                        qkT_sb[:, nq + kv, sc * 128:(sc + 1) * 128],
                        qkT_sb[:, qh, l0 + off:l0 + 512],
                        start=True, stop=True,
                    )
                    ex = expp.tile([128, 512], BF16, tag="exp")
                    nc.scalar.activation(
                        out=ex[:, 0:w], in_=ps[:, 0:w],
                        func=mybir.ActivationFunctionType.Exp, scale=SCALE,
                    )
                    if causal and sc * 128 > l0 - 128:
                        # keep where s0+p <= l0+off+f; base is always 0 here
                        # so only the leading 128 columns can be masked --
                        # beyond f=128 > p_max the predicate always holds.
                        # During the B-interleaved groups Pool is busy with
                        # k-rope, so mask there via the DVE triangle multiply
                        mw = min(w, 128 - (l0 + off - sc * 128))
                        if lqb <= 2:
                            nc.vector.tensor_mul(
                                ex[:, 0:mw], ex[:, 0:mw], tri_sb[:, 0:mw]
                            )
                        else:
                            nc.gpsimd.affine_select(
                                out=ex[:, 0:mw], in_=ex[:, 0:mw],
                                pattern=[[1, mw]],
                                compare_op=mybir.AluOpType.is_ge, fill=0.0,
                                base=l0 + off - sc * 128,
                                channel_multiplier=-1,
                            )
                    if not causal:
                        nc.vector.tensor_mul(ex, ex, mrows[:, sc, :])
                    # denominator accumulation on DVE; the first two full
                    # chunks fuse into a single 3-operand add
                    if sc == 0:
                        if n_sc == 1 or chunk_off(1) != 0:
                            nc.vector.tensor_copy(out=exsum, in_=ex)
                    elif sc == 1 and off == 0:
                        nc.vector.tensor_add(exsum, exps[0][0], ex)
                    else:
                        nc.vector.tensor_add(
                            exsum[:, off:512], exsum[:, off:512],
                            ex[:, 0:w],
                        )
                    exps.append((ex[:, 0:w], sc, off))
                    if sc >= DEPTH:
                        av_mm(*exps[sc - DEPTH])
                    if sc == 1:
                        run_fin_a()  # previous head's den sum + reciprocal
                    elif sc == max(2, min(4, n_sc - 1)):
                        run_fin_b()  # previous head's broadcast + normalize
                    if d_iter is not None and sc % d_every == d_every - 1:
                        emit_d_unit(d_iter, d_engs)
                for sc in range(max(0, n_sc - DEPTH), n_sc):
                    av_mm(*exps[sc])
                if n_sc <= 1:
                    run_fin_b()

                # denominator: one ones-matmul over the accumulated exsum
                # (vs one per chunk), reciprocal, then a PE partition
                # broadcast in f32r and the final normalize on DVE.
                # Deferred in two stages into the next head's chunk stream so
                # neither the scores-ring slots nor the DVE chain stall PE.
                fstate = {}

                def fin_a():
                    if DEN_MODE == "pool":
                        den = stats.tile([1, 512], F32, tag="den", bufs=2)
                        nc.gpsimd.tensor_reduce(
                            den, exsum, mybir.AxisListType.C,
                            mybir.AluOpType.add,
                        )
                    else:
                        den = psum.tile([1, 512], F32, tag="proj",
                                        bufs=3,
                                        name=f"pden_{qh}_{lqb}")
                        nc.tensor.matmul(den, ones_sb, exsum,
                                         start=True, stop=True)
                    rden = stats.tile([1, 512], F32R, tag="rden", bufs=2)
                    with nc.allow_low_precision(
                        reason="f32r reciprocal keeps full fp32 bits"
                    ):
                        nc.vector.reciprocal(out=rden, in_=den)
                    fstate["rden"] = rden

                def fin_b():
                    pbc = psum.tile([128, 512], F32, tag="scores",
                                    bufs=SCORES_BUFS, name=f"bc_{qh}_{lqb}")
                    nc.tensor.matmul(pbc, onesf_sb, fstate["rden"],
                                     start=True, stop=True)
                    rdenb = work.tile([128, 512], F32, tag="rdenb", bufs=2)
                    nc.vector.tensor_copy(out=rdenb, in_=pbc)
                    nc.vector.tensor_mul(
                        qkvT_sb[:, qh, l0:l0 + 512], pav, rdenb
                    )

                cstate["fin_a"] = fin_a
                cstate["fin_b"] = fin_b

            def c_block(lqb, d_iter=None, d_every=2,
                        d_engs=("vector", "scalar"), last_engs=None):
                for qh in range(nq):
                    eng = (last_engs if (last_engs and qh == nq - 1)
                           else d_engs)
                    c_head(lqb, qh, d_iter, d_every, eng)

            # ---- last output-projection group, hh-outer so only the final
            # 4 matmuls per wave wait on the last head's qkvT write ----
            d3_lbs = list(range(4 * (n_lqb - 1), n_lb))
            D3_TAGS = [("proj", 3), ("proj", 3), ("proj", 3),
                       ("scores", SCORES_BUFS)]

            def d3_wave_alloc(db):
                pos = {}
                for i, lb in enumerate(d3_lbs):
                    tg, bf = D3_TAGS[i % len(D3_TAGS)]
                    pos[lb] = psum.tile([128, 512], F32, tag=tg, bufs=bf,
                                        name=f"po3_{lb}_{db}")
                return pos

            def d3_mm(pos, db, hh_range):
                for hh in hh_range:
                    for lb in d3_lbs:
                        nc.tensor.matmul(
                            pos[lb],
                            qkvT_sb[:, hh, lb * 128:(lb + 1) * 128],
                            wo_sb[:, hh, db * 512:(db + 1) * 512],
                            start=(hh == 0), stop=(hh == nq - 1),
                            skip_group_check=True,
                        )

            def d3_finish_wave(pos, db):
                for i, lb in enumerate(d3_lbs):
                    oslab = outp.tile([128, 512], BF16, tag="ot3", bufs=6,
                                      name=f"ot3_{lb}_{db}")
                    _copy(nc, ("vector", "scalar")[i % 2], oslab, pos[lb])
                    nc.sync.dma_start(
                        out=out_d[lb * 128:(lb + 1) * 128,
                                  db * 512:(db + 1) * 512],
                        in_=oslab,
                    )

            def d3_fused(pre=None):
                for db in range(n_db):
                    if pre is not None and db == 0:
                        pos = pre
                        d3_mm(pos, db, range(nq - 1, nq))
                    else:
                        pos = d3_wave_alloc(db)
                        d3_mm(pos, db, range(nq))
                    d3_finish_wave(pos, db)

            # ---- schedule ----
            if INTERLEAVE == "fine" and causal and n_lqb == 4 and n_lb == 16:
                xts = preload_startup()
                b_fused_start({k: v for k, v in xts.items() if k < 3})
                for lb in range(3, 9):
                    b_block(lb, xts.get(lb))
                if WO_LATE:
                    load_wo()
                del xts
                # C0/C1 head-units between the remaining B blocks: PE keeps
                # crunching proj matmuls while ACT chews the C exps.  The
                # last C1 heads (issued after every B block) already pull in
                # D0 units to plug PE bubbles from the exp backlog.
                it01 = d_iter_for([0, 1])
                cunits = ([(0, q, None) for q in range(nq)]
                          + [(1, 0, None), (1, 1, None),
                             (1, 2, it01), (1, 3, it01)])
                rest_b = list(range(9, n_lb))
                while cunits or rest_b:
                    if rest_b:
                        b_block(rest_b.pop(0))
                    if cunits:
                        lqb, qh, dit = cunits.pop(0)
                        c_head(lqb, qh, dit, d_every=2)
                # transposes for lb 13..15 stay pending: C2 needs only lb<=11;
                # pop them one per C2 head so their copies spread out
                for qh in range(nq):
                    c_head(2, qh, it01, d_every=2)
                    if state["pending"]:
                        transpose_block(*state["pending"].pop(0))
                flush_pending()
                drain_d(it01)
                it2 = d_iter_for([2])
                c_block(3, it2, d_every=4, d_engs=("vector",))
                # wave-0 partial sums for heads 0..2 keep PE busy while the
                # last head's denominator chain drains
                pre = d3_wave_alloc(0)
                d3_mm(pre, 0, range(nq - 1))
                run_fin_b()
                drain_d(it2, engs=("vector", "scalar"))
                d3_fused(pre)
            else:
                xts = preload_startup()
                for lb in range(n_lb):
                    b_block(lb, xts.get(lb))
                if WO_LATE:
                    load_wo()
                for g in range(n_lqb):
                    if g == n_lqb - 1 or not causal:
                        flush_pending()
                    c_block(g)
                    flush_pending()
                run_fin_b()
                drain_d(d_iter_for(list(range(n_lqb))))
    return nc


# ---------------- host side ----------------

def _x_block(xb, L_=L, D_=D):
    """Host-preblocked x: [lb, p, dc*128] with element x[lb*128+l, dc*128+p]."""
    n_lb, n_dc = L_ // 128, D_ // 128
    y = xb.reshape(n_lb, 128, n_dc, 128).transpose(0, 3, 2, 1)
    return np.ascontiguousarray(y).reshape(n_lb, 128, D_)


def _rope_block(pos, qw, kw, L_=L):
    """Host-preblocked rope tables: [n_grp, 128, grp*8*(H//2)] bf16 where
    element [gi, p, (Bi, a, j)] = table[a, (gi*grp+Bi)*128 + p, j]."""
    tabs = np.concatenate([_rope_tables(pos, qw), _rope_tables(pos, kw)])
    n_lb = L_ // 128
    grp = 4 if n_lb % 4 == 0 else 1
    t = tabs.reshape(8, n_lb // grp, grp, 128, H // 2)
    return np.ascontiguousarray(t.transpose(1, 3, 2, 0, 4)).reshape(
        n_lb // grp, 128, grp * 8 * (H // 2)
    ).astype(ml_dtypes.bfloat16)


def _rope_tables(pos, norm_w):
    """A,B,C,D [4, L, H/2] f32 with the rms-norm weight folded in.
    h1 = q1*A - q2*B ; h2 = q2*C + q1*D  (q already divided by rms)."""
    hh = H // 2
    fraction = 2.0 * np.arange(hh, dtype=np.float32) / np.float32(H)
    timescale = np.float32(ROPE_THETA) ** fraction
    sinusoid = pos.astype(np.float32)[:, None] / timescale[None, :]
    sin = np.sin(sinusoid).astype(np.float32)
    cos = np.cos(sinusoid).astype(np.float32)
    w1 = norm_w[:hh].astype(np.float32)
    w2 = norm_w[hh:].astype(np.float32)
    return np.stack([cos * w1, sin * w2, cos * w2, sin * w1]).astype(np.float32)


_KERNELS = {}
TRACE = False
LAST_RESULTS = None


def _get_kernel(causal):
    if causal not in _KERNELS:
        _KERNELS[causal] = build_core_kernel(causal=causal)
    return _KERNELS[causal]


def kernel(**inputs):
    x = np.asarray(inputs["x"], dtype=np.float32)
    pos = np.asarray(inputs["position_ids"])
    mask = np.asarray(inputs["attn_mask"]).astype(bool)
    wq = np.asarray(inputs["wq"], dtype=np.float32)
    wk = np.asarray(inputs["wk"], dtype=np.float32)
    wv = np.asarray(inputs["wv"], dtype=np.float32)
    wo = np.asarray(inputs["wo"], dtype=np.float32)
    qw = np.asarray(inputs["q_norm_w"], dtype=np.float32)
    kw = np.asarray(inputs["k_norm_w"], dtype=np.float32)

    tril = np.tril(np.ones((L, L), dtype=bool))
    causal = all(np.array_equal(mask[b], tril) for b in range(B))
    nc = _get_kernel(causal)

    bf = ml_dtypes.bfloat16
    per_batch = []
    for b in range(B):
        d = {
            "xT": _x_block(x[b].astype(bf)),
            "rope": _rope_block(pos[b], qw, kw),
        }
        if not causal:
            d["maskT"] = np.ascontiguousarray(mask[b].T).astype(bf)
        per_batch.append(d)

    in_maps = []
    for c in range(N_CORES):
        b, g = divmod(c, N_CORES // B)
        qs = slice(QH_PER_CORE * g, QH_PER_CORE * (g + 1))
        ks = slice(KV_PER_CORE * g, KV_PER_CORE * (g + 1))
        wqkv = np.concatenate(
            [
                wq[:, qs, :].reshape(D, QH_PER_CORE * H),
                wk[:, ks, :].reshape(D, KV_PER_CORE * H),
                wv[:, ks, :].reshape(D, KV_PER_CORE * H),
            ],
            axis=1,
        ).astype(bf)
        m = dict(per_batch[b])
        m["wqkv"] = wqkv
        m["wo"] = np.ascontiguousarray(wo[qs].reshape(QH_PER_CORE * H, D)).astype(bf)
        in_maps.append(m)

    global LAST_RESULTS
    res = run_bass_kernel_spmd(
        nc, in_maps, core_ids=list(range(N_CORES)), trace=TRACE
    )
    LAST_RESULTS = res
    out = np.zeros((B, L, D), dtype=np.float32)
    for c in range(N_CORES):
        out[c // (N_CORES // B)] += res.results[c]["out"]
    return out



# revision 116
# speedup vs baseline: 1.0041x; 1.0017x over previous
"""Trainium2 Bass kernel for a GQA attention block (B=2, L=2048, D=2048,
16 q-heads / 8 kv-heads, head_dim=128), sharded over 8 NeuronCores.

Sharding: core c -> batch b = c // 4, head-group g = c % 4 (4 q-heads and
their 2 kv-heads).  Each core computes its heads' attention plus the partial
output projection; the host sums the 4 partials per batch.

Self-contained: only needs numpy / ml_dtypes / concourse (on PYTHONPATH in
this container).
"""

import math
import sys

for _p in ("/root/.axon_site", "/root/.axon_site/_ro/trn_rl_repo",
           "/root/.axon_site/_ro/pypackages"):
    if _p not in sys.path:
        sys.path.append(_p)

import numpy as np
import ml_dtypes

import concourse.bass as bass
import concourse.bass2jax as bass2jax
import concourse.bass_isa as bass_isa
import concourse.mybir as mybir
import concourse.tile as tile
from concourse.masks import make_identity
from concourse.bass_utils import run_bass_kernel_spmd
from concourse.vector_clock import ScopedClock, VectorClock


def _legalize_bir_waits(bir_bytes):
    """This walrus build supports only ONE sync-wait slot per instruction.
    Hoist extra waits onto NoOp instructions inserted just before the
    offender (same engine, so the engine stream still blocks in order)."""
    import orjson

    d = orjson.loads(bir_bytes)
    n_split = 0
    for f in d["functions"]:
        for bb in f["blocks"]:
            out = []
            for inst in bb["instructions"]:
                si = inst.get("sync_info")
                waits = (si or {}).get("on_wait") or []
                if len(waits) > 1:
                    for j, w in enumerate(waits[:-1]):
                        n_split += 1
                        out.append({
                            "engine": inst["engine"], "ins": [], "outs": [],
                            "name": f"{inst['name']}__w{j}",
                            "opcode": "NoOp",
                            "sync_info": {"on_wait": [w], "on_update": []},
                        })
                    si["on_wait"] = [waits[-1]]
                out.append(inst)
            bb["instructions"] = out
    return orjson.dumps(d)


_orig_compile_bir_kernel = bass2jax.compile_bir_kernel


def _patched_compile_bir_kernel(ant_bir_str, *args, **kwargs):
    return _orig_compile_bir_kernel(_legalize_bir_waits(ant_bir_str), *args, **kwargs)


bass2jax.compile_bir_kernel = _patched_compile_bir_kernel

BF16 = mybir.dt.bfloat16
F32 = mybir.dt.float32

# Full-problem constants
B, L, D = 2, 2048, 2048
N_HEADS, N_KV, H = 16, 8, 128
EPS = 1e-6
ROPE_THETA = 1e6
N_CORES = 8
QH_PER_CORE = N_HEADS // (N_CORES // B)   # 4
KV_PER_CORE = N_KV // (N_CORES // B)      # 2
SCALE = H ** -0.5


class PatchedTileContext(tile.TileContext):
    """This walrus build only supports one sync-wait slot on a CTRL (Drain)
    instruction; split the tail-drain waits across one drain per processor."""

    def _drain_and_barrier(self, tick_clock, wait_clock):
        gc = tick_clock.global_clock
        n = len(gc)
        for p in range(n):
            t = gc[p]
            if t > 0:
                vc = VectorClock([t if i == p else 0 for i in range(n)])
                d = self.nc.sync.drain()
                wait_clock.add_sem_waits(d.ins, ScopedClock({None: vc}))
                si = d.ins.sync_info
                nw = len(si.on_wait) if si is not None else 0
                assert nw <= 1, f"proc {p} produced {nw} waits"
        self.nc.all_engine_barrier()
        assert self.sems is not None
        popped = self.nc._tile_sem_poison_stack.pop()
        assert popped is self._sem_poison
        self.nc.clear_and_free_semaphores(list(self.sems.allocated().values()))
        self.nc.all_engine_barrier()


# engine assignment knobs (tuned against the cost-model timeline)
TCOPY_ENG = "scalar"   # transpose PSUM->SBUF copies (gpsimd cannot touch PSUM)
VCOPY_ENG = "vector"  # v PSUM->SBUF copy
OCOPY_SPLIT = True     # split phase-D copies between DVE and ACT
DELAY_T = True         # transpose qkn one L-block late
ROPE_STT = True        # fold rstd into stt ops reading the SBUF staging
WQKV_SPLIT = True      # split the wqkv load into 4 chunks after xt(0)
WO_LATE = True         # load wo just before it's needed
INTERLEAVE = "fine"    # "fine": B/C head interleave + C/D unit interleave
PE_WARMUP = 0
QKRAW_ENG = "scalar"   # staging copies of q+k PSUM->SBUF
C_DEPTH = 5            # attention chunk software-pipeline depth
EXP_BUFS = 7
ROPE_K_ENG = "gpsimd"  # engine for k-head rope multiplies
WQKV_FIRST1 = False    # make the first wqkv chunk a single dc
STATS_BUFS = 6
WORK_BUFS = 3
SCORES_BUFS = 3        # scores PSUM ring (av 3 + scores 3 + proj 2 = 8 banks)
DEN_MODE = "pool"      # "pool": gpsimd C-axis reduce; "mm": PE ones-matmul


def _copy(nc, eng, out, in_):
    if eng == "vector":
        nc.vector.tensor_copy(out=out, in_=in_)
    elif eng == "gpsimd":
        nc.gpsimd.tensor_copy(out=out, in_=in_)
    else:
        nc.scalar.copy(out=out, in_=in_)


def build_core_kernel(L_=L, D_=D, nq=QH_PER_CORE, nkv=KV_PER_CORE, causal=True):
    """One core's program.  Inputs (DRAM):
      xT    [L/128, 128, D] bf16 — host-preblocked x (see _x_block)
      wqkv  [D, nq*H + 2*nkv*H] bf16  ([wq heads | wk heads | wv heads])
      wo    [nq*H, D] bf16
      rope  [L/128/G, 128, G*8*(H/2)] f32 — preblocked A,B,C,D cos/sin
            tables for q then k, norm weights folded in (see _rope_block)
      maskT [L, L] bf16 (only if causal=False; 0/1 multiplicative, [s, l])
    Output:
      out [L, D] bf16 — partial sum over this core's heads (the host
      accumulates the four per-batch partials in f32).

    Layout strategy: projections produce q/k/v in natural [L-part, H] layout
    (RMS-norm + RoPE are row-wise there, all bf16 so DVE gets its 2x/4x
    modes; k-head rope multiplies run on gpsimd), q/k are transposed per
    128-block on the PE (identity matmul); attention runs fully transposed —
    scores^T = kT^T qT, exp on ACT, causal mask via a 128-col affine_select
    (gpsimd) or triangle multiply (DVE), av^T accumulated over S-chunks in
    PSUM — so av^T feeds the output projection as lhsT with zero further
    transposes.  The softmax denominator is a bf16 running sum of the exp
    tiles on DVE, reduced across partitions once per (head, q-group) by a
    gpsimd C-axis tensor_reduce, then broadcast by a tiny f32r PE matmul;
    the whole finalize is deferred into the next head's chunk stream.
    Schedule: dc-interleaved fused start for blocks 0-2 chasing the
    dc-ordered wqkv/x DMA queue, C0/C1 head-units between the later B
    blocks, output-projection (lb,db)-units woven into C2/C3's chunk
    streams, and an hh-outer fused final D group.
    """
    HH = H // 2
    n_lb = L_ // 128          # L blocks of 128
    n_dc = D_ // 128          # D contraction chunks
    n_lqb = L_ // 512         # q blocks of 512
    QCOLS = nq * H
    KCOLS = nkv * H
    KV_COLS = 2 * nkv * H
    W_COLS = QCOLS + KV_COLS
    assert W_COLS % 512 == 0
    n_wslab = W_COLS // 512   # 512-wide slabs of the qkv projection

    nc = bass.Bass()
    # x, host-preblocked: [lb, p, dc*128] with element = x[lb*128+l, dc*128+p]
    xT_d = nc.dram_tensor("xT", [L_ // 128, 128, D_], BF16, kind="ExternalInput")
    wqkv_d = nc.dram_tensor("wqkv", [D_, W_COLS], BF16, kind="ExternalInput")
    wo_d = nc.dram_tensor("wo", [QCOLS, D_], BF16, kind="ExternalInput")
    # rope tables, host-preblocked: [group, p, (lb-in-group, table 0..7, j)]
    ROPE_GRP = 4 if (L_ // 128) % 4 == 0 else 1
    rope_d = nc.dram_tensor(
        "rope", [L_ // 128 // ROPE_GRP, 128, ROPE_GRP * 8 * HH], BF16,
        kind="ExternalInput",
    )
    if not causal:
        maskT_d = nc.dram_tensor("maskT", [L_, L_], BF16, kind="ExternalInput")
    out_d = nc.dram_tensor("out", [L_, D_], BF16, kind="ExternalOutput")
    nqk = nq + nkv  # q heads then k heads in the combined transposed tile

    with PatchedTileContext(nc) as tc:
        with (
            tc.tile_pool(name="res", bufs=1) as res,
            tc.tile_pool(name="ropetab", bufs=3) as ropetab,
            tc.tile_pool(name="work", bufs=WORK_BUFS) as work,
            tc.tile_pool(name="stats", bufs=STATS_BUFS) as stats,
            tc.tile_pool(name="expp", bufs=EXP_BUFS) as expp,
            tc.tile_pool(name="outp", bufs=3) as outp,
            tc.tile_pool(name="psum", bufs=1, space="PSUM") as psum,
            tc.tile_pool(name="maskp", bufs=2) as maskp,
        ):
            # ---- resident loads ----
            wqkv_sb = res.tile([128, n_dc, W_COLS], BF16, tag="wqkv")
            wqkv_r = wqkv_d.rearrange("(dc p) c -> p dc c", p=128)
            wo_sb = res.tile([128, nq, D_], BF16, tag="wo")

            def load_wo():
                nc.scalar.dma_start(
                    out=wo_sb, in_=wo_d.rearrange("(hh p) d -> p hh d", p=128)
                )

            if not WO_LATE:
                load_wo()
            ones_sb = res.tile([128, 1], BF16, tag="ones")
            nc.vector.memset(ones_sb, 1.0)
            # f32r ones row: the partition-broadcast matmul runs at bf16
            # speed for N>=256 while keeping full fp32 mantissa bits
            F32R = mybir.dt.float32r
            onesf_sb = res.tile([1, 128], F32R, tag="onesf")
            onesf_f32 = res.tile([1, 128], F32, tag="onesf32")
            nc.vector.memset(onesf_f32, 1.0)
            with nc.allow_low_precision(reason="f32r ones"):
                nc.vector.tensor_copy(out=onesf_sb, in_=onesf_f32)
            eps_sb = res.tile([128, 1], F32, tag="eps")
            nc.vector.memset(eps_sb, EPS)
            ident_sb = res.tile([128, 128], BF16, tag="ident")
            make_identity(nc, ident_sb)
            # 0/1 lower-triangle (keep f >= p) for the causal mask multiply
            tri_sb = res.tile([128, 128], BF16, tag="tri")
            nc.vector.memset(tri_sb, 1.0)
            nc.gpsimd.affine_select(
                out=tri_sb, in_=tri_sb, pattern=[[1, 128]],
                compare_op=mybir.AluOpType.is_ge, fill=0.0,
                base=0, channel_multiplier=-1,
            )
            if PE_WARMUP:
                pw = psum.tile([128, 128], BF16, tag="scores", bufs=3,
                               name="pe_warm")
                for _ in range(PE_WARMUP):
                    nc.tensor.transpose(pw, ident_sb, ident_sb)

            v_sb = res.tile([128, n_lb, KCOLS], BF16, tag="v")
            qkT_sb = res.tile([128, nqk, L_], BF16, tag="qkT")
            qkvT_sb = res.tile([128, nq, L_], BF16, tag="qkvT")

            # ---- phase B: qkv projection + rmsnorm + rope + transposes ----
            def head_stats(src, ssq6, h, eng="vector"):
                """sum(x^2) for one head: stt square with accumulator
                (x*1)*x; DVE gets it cheap in bf16, but the fused-start
                blocks use ACT Square to spare DVE's post-fused burst."""
                sq = work.tile([128, H], BF16, tag="sq")
                if eng == "act":
                    nc.scalar.activation(
                        out=sq, in_=src,
                        func=mybir.ActivationFunctionType.Square,
                        accum_out=ssq6[:, h:h + 1],
                    )
                else:
                    nc.vector.scalar_tensor_tensor(
                        out=sq, in0=src, scalar=1.0, in1=src,
                        op0=mybir.AluOpType.mult, op1=mybir.AluOpType.mult,
                        accum_out=ssq6[:, h:h + 1],
                    )

            def finish_stats(ssq6, rstd6):
                # one fused sqrt / reciprocal for all heads of the block
                nc.scalar.activation(
                    out=rstd6[:, 0:nqk], in_=ssq6[:, 0:nqk],
                    func=mybir.ActivationFunctionType.Sqrt,
                    bias=eps_sb, scale=1.0 / H,
                )
                nc.vector.reciprocal(out=rstd6[:, 0:nqk], in_=rstd6[:, 0:nqk])

            def rope_head(src, rstd, rtab, qkn, dcol):
                """RoPE one head (src: [128, H] bf16 in SBUF); bf16 for DVE
                2x/4x perf modes:
                h1 = (q1*rstd)*A - (q2*rstd)*B ; h2 = (q2*rstd)*C + (q1*rstd)*D
                k heads run their multiplies on gpsimd (plain tensor_tensor
                only -- Pool has no stt opcode), with the rstd prescale done
                on DVE where tensor_scalar gets the 4x mode."""
                qb = qkn[:, dcol * H:(dcol + 1) * H]
                mul = mybir.AluOpType.mult
                on_pool = dcol >= nq and ROPE_K_ENG == "gpsimd"
                veng = nc.gpsimd if on_pool else nc.vector
                if on_pool:
                    qn = work.tile([128, H], BF16, tag="qn")
                    nc.vector.tensor_scalar_mul(qn, src, rstd)
                    s1, s2 = qn[:, 0:HH], qn[:, HH:H]
                else:
                    s1, s2 = src[:, 0:HH], src[:, HH:H]
                # separate tile rings per engine: sharing them would make
                # DVE waits ride on Pool's slower ops via slot reuse
                tga, tgb = ("kt1", "kt2") if on_pool else ("t1", "t2")
                t1 = work.tile([128, HH], BF16, tag=tga, name="t1")
                t2 = work.tile([128, HH], BF16, tag=tgb, name="t2")

                def rmul(out, sx, tab):
                    if on_pool:
                        veng.tensor_mul(out, sx, tab)
                    else:
                        veng.scalar_tensor_tensor(
                            out=out, in0=sx, scalar=rstd, in1=tab,
                            op0=mul, op1=mul)

                rmul(t1, s1, rtab[:, 0, :])
                rmul(t2, s2, rtab[:, 1, :])
                veng.tensor_sub(qb[:, 0:HH], t1, t2)
                t3 = work.tile([128, HH], BF16, tag=tga, name="t3")
                t4 = work.tile([128, HH], BF16, tag=tgb, name="t4")
                rmul(t3, s2, rtab[:, 2, :])
                rmul(t4, s1, rtab[:, 3, :])
                veng.tensor_add(qb[:, HH:H], t3, t4)

            def transpose_block(qkn, lb):
                # transpose each head block on PE (identity matmul)
                for h in range(nqk):
                    pt = psum.tile([128, 128], BF16, tag="scores", bufs=SCORES_BUFS,
                                   name=f"pt_{lb}_{h}")
                    nc.tensor.transpose(
                        pt, qkn[:, h * H:(h + 1) * H], ident_sb
                    )
                    _copy(nc, TCOPY_ENG,
                          qkT_sb[:, h, lb * 128:(lb + 1) * 128], pt)

            state = {"pending": []}  # [(qkn, lb)] transposed T_DELAY late

            def preload_startup():
                """Interleave the wqkv chunks with xt pieces for blocks 0/1
                in dc order on ONE queue, so early proj matmuls are gated by
                the minimum prefix of bytes rather than whole-tensor DMAs."""
                if not (WQKV_SPLIT and n_dc >= 8):
                    nc.scalar.dma_start(out=wqkv_sb, in_=wqkv_r)
                    return {}
                xts = {}
                for i in range(min(4, n_lb)):
                    t = work.tile([128, n_dc, 128], BF16, tag="xt", bufs=4,
                                  name=f"xt_{i}")
                    xts[i] = t
                # weight chunks in dc order; xt pieces (>=2 dc so each DMA
                # descriptor stays >=512B) slotted between them
                # block 0's first piece goes down the SP queue so its DMA
                # latency chain overlaps the first weight chunk's
                nc.sync.dma_start(out=xts[0][:, 0:2, :],
                                  in_=xT_d[0, :, 0:256])
                wb = [0, 1, 2, 3, 4, 6, 8, 10, 12, 14, n_dc]
                xb = [0, 2, 4, 6, 8, 10, 12, 14, n_dc]
                xi = 0
                for i, j in zip(wb[:-1], wb[1:]):
                    nc.scalar.dma_start(
                        out=wqkv_sb[:, i:j, :], in_=wqkv_r[:, i:j, :]
                    )
                    while xi + 1 < len(xb) and xb[xi + 1] <= j:
                        a, b_ = xb[xi], xb[xi + 1]
                        for bi, t in xts.items():
                            if bi == 3 or (bi == 0 and a == 0):
                                continue  # block 3 loads after the hot path
                            nc.scalar.dma_start(
                                out=t[:, a:b_, :],
                                in_=xT_d[bi, :, a * 128:b_ * 128],
                            )
                        xi += 1
                if 3 in xts:
                    nc.scalar.dma_start(out=xts[3], in_=xT_d[3])
                return xts

            def b_fused_start(xts):
                """Projection matmuls for blocks 0..2 interleaved by dc so
                PE consumption tracks the dc-ordered DMA arrival; one PSUM
                accumulator pair per block drawn from the three tag rings."""
                tags = [("proj", 3), ("scores", SCORES_BUFS), ("av", 2)]
                pqs = {}
                for bi in xts:
                    tg, bf = tags[bi]
                    pqs[bi] = [psum.tile([128, 512], F32, tag=tg, bufs=bf,
                                         name=f"projf_{bi}_{s}")
                               for s in range(n_wslab)]
                for dc in range(n_dc):
                    for bi, t in xts.items():
                        for s in range(n_wslab):
                            nc.tensor.matmul(
                                pqs[bi][s],
                                t[:, dc, :],
                                wqkv_sb[:, dc, s * 512:(s + 1) * 512],
                                start=(dc == 0), stop=(dc == n_dc - 1),
                                skip_group_check=True,
                            )
                for bi in xts:
                    b_post(bi, pqs[bi])

            def b_block(lb, xt_pre=None):
                if xt_pre is not None:
                    xt = xt_pre
                else:
                    xt = work.tile([128, n_dc, 128], BF16, tag="xt", bufs=4,
                                   name=f"xt_{lb}")
                    nc.sync.dma_start(out=xt, in_=xT_d[lb])
                # dc-outer: both slabs accumulate in parallel, so early
                # weight chunks enable matmuls in arrival order
                pqs = [psum.tile([128, 512], F32, tag="proj", bufs=3,
                                 name=f"proj_{lb}_{s}")
                       for s in range(n_wslab)]
                for dc in range(n_dc):
                    for s in range(n_wslab):
                        nc.tensor.matmul(
                            pqs[s],
                            xt[:, dc, :],
                            wqkv_sb[:, dc, s * 512:(s + 1) * 512],
                            start=(dc == 0), stop=(dc == n_dc - 1),
                            skip_group_check=True,
                        )
                b_post(lb, pqs)

            def b_post(lb, pqs):
                # bulk-stage q+k to SBUF (bf16 so rope DVE ops get 2x/4x);
                # v goes straight to its resident tile
                qkraw = work.tile([128, QCOLS + KCOLS], BF16, tag="qkraw",
                                  bufs=3, name=f"qkraw_{lb}")
                off = 0
                for s in range(n_wslab):
                    w = min(512, QCOLS + KCOLS - off)
                    if w > 0:
                        _copy(nc, QKRAW_ENG,
                              qkraw[:, off:off + w], pqs[s][:, 0:w])
                    off += 512
                vt, voff = pqs[(QCOLS + KCOLS) // 512], (QCOLS + KCOLS) % 512
                _copy(nc, VCOPY_ENG, v_sb[:, lb, :],
                      vt[:, voff:voff + KCOLS])

                if lb % ROPE_GRP == 0:
                    state["rope_t"] = ropetab.tile(
                        [128, ROPE_GRP, 8, HH], BF16, tag="rope", bufs=2,
                        name=f"rope_{lb}")
                    nc.scalar.dma_start(
                        out=state["rope_t"], in_=rope_d[lb // ROPE_GRP],
                    )
                rope_t = state["rope_t"]
                rq = rope_t[:, lb % ROPE_GRP, 0:4, :]
                rk = rope_t[:, lb % ROPE_GRP, 4:8, :]

                qkn = work.tile([128, nqk * H], BF16, tag="qkn", bufs=5,
                                name=f"qkn_{lb}")
                ssq6 = stats.tile([128, 8], F32, tag="ssq6")
                rstd6 = stats.tile([128, 8], F32, tag="rstd6")
                for h in range(nqk):
                    head_stats(qkraw[:, h * H:(h + 1) * H], ssq6, h)
                finish_stats(ssq6, rstd6)
                for h in range(nqk):
                    rope_head(
                        qkraw[:, h * H:(h + 1) * H], rstd6[:, h:h + 1],
                        rq if h < nq else rk, qkn, h,
                    )
                depth = 3 if DELAY_T else 0
                state["pending"].append((qkn, lb))
                while len(state["pending"]) > depth:
                    transpose_block(*state["pending"].pop(0))

            def flush_pending():
                while state["pending"]:
                    transpose_block(*state["pending"].pop(0))

            # ---- phase D units: one (lb, db) output-projection tile ----
            n_db = D_ // 512
            dstate = {"ot": None, "ocount": 0}

            def emit_d_unit(it, engs=("vector", "scalar")):
                try:
                    lb, db = next(it)
                except StopIteration:
                    return False
                if db == 0:
                    ot_tile = outp.tile([128, D_], BF16, tag="ot",
                                        name=f"ot_{lb}")
                    dstate["ot"] = ot_tile
                ot = dstate["ot"]
                po = psum.tile([128, 512], F32, tag="proj", bufs=3,
                               name=f"po_{lb}_{db}")
                for hh in range(nq):
                    nc.tensor.matmul(
                        po,
                        qkvT_sb[:, hh, lb * 128:(lb + 1) * 128],
                        wo_sb[:, hh, db * 512:(db + 1) * 512],
                        start=(hh == 0), stop=(hh == nq - 1),
                        skip_group_check=True,
                    )
                eng = (engs[dstate["ocount"] % len(engs)]
                       if OCOPY_SPLIT else "vector")
                dstate["ocount"] += 1
                oslab = ot[:, db * 512:(db + 1) * 512]
                if eng == "gpsimd":
                    nc.gpsimd.tensor_copy(out=oslab, in_=po)
                else:
                    _copy(nc, eng, oslab, po)
                if lb == n_lb - 1:
                    # split the very last row-block's DMA to shorten the tail
                    nc.sync.dma_start(
                        out=out_d[lb * 128:(lb + 1) * 128,
                                  db * 512:(db + 1) * 512],
                        in_=oslab,
                    )
                elif db == n_db - 1:
                    nc.sync.dma_start(
                        out=out_d[lb * 128:(lb + 1) * 128, :], in_=ot,
                    )
                return True

            def d_iter_for(gs):
                return iter([(lb, db)
                             for g in gs
                             for lb in range(4 * g, min(4 * g + 4, n_lb))
                             for db in range(n_db)])

            def drain_d(it, engs=("vector", "scalar")):
                while emit_d_unit(it, engs):
                    pass

            # ---- phase C: attention for one (head, 512-wide q group) ----
            cstate = {"fin_a": None, "fin_b": None}

            def run_fin_a():
                if cstate["fin_a"] is not None:
                    cstate["fin_a"]()
                    cstate["fin_a"] = None

            def run_fin_b():
                run_fin_a()
                if cstate["fin_b"] is not None:
                    cstate["fin_b"]()
                    cstate["fin_b"] = None

            def c_head(lqb, qh, d_iter=None, d_every=2,
                       d_engs=("vector", "scalar")):
                l0 = lqb * 512
                n_sc = min(n_lb, (l0 + 512) // 128) if causal else n_lb
                kv = qh // (nq // nkv)
                pav = psum.tile([128, 512], F32, tag="av", bufs=2,
                                name=f"av_{qh}_{lqb}")
                # running sum of exp rows (softmax denominator), built on
                # DVE so no PE ones-matmul per chunk is needed
                exsum = expp.tile([128, 512], BF16, tag="exsum", bufs=3,
                                  name=f"exsum_{qh}_{lqb}")
                if not causal:
                    mrows = maskp.tile([128, n_lb, 512], BF16, tag="mask")
                    nc.scalar.dma_start(
                        out=mrows,
                        in_=maskT_d[:, l0:l0 + 512].rearrange(
                            "(sb p) l -> p sb l", p=128
                        ),
                    )
                exps = []

                def av_mm(ex, sc, off):
                    # diagonal chunks only have live columns f >= s0-l0
                    nc.tensor.matmul(
                        pav[:, off:512],
                        v_sb[:, sc, kv * H:(kv + 1) * H], ex,
                        start=(sc == 0), stop=(sc == n_sc - 1),
                        skip_group_check=True,
                    )

                # software pipeline: av(c-DEPTH) after qk(c) so the
                # exp+mask latency of chunk c never stalls PE
                DEPTH = C_DEPTH if n_sc > C_DEPTH else max(1, n_sc - 1)

                def chunk_off(sc):
                    return max(0, sc * 128 - l0) if causal else 0

                for sc in range(n_sc):
                    off = chunk_off(sc)
                    w = 512 - off
                    ps = psum.tile([128, 512], F32, tag="scores",
                                   bufs=SCORES_BUFS,
                                   name=f"sc_{qh}_{lqb}_{sc}")
                    nc.tensor.matmul(
                        ps[:, 0:w],
                        qkT_sb[:, nq + kv, sc * 128:(sc + 1) * 128],
                        qkT_sb[:, qh, l0 + off:l0 + 512],
                        start=True, stop=True,
                    )
                    ex = expp.tile([128, 512], BF16, tag="exp")
                    nc.scalar.activation(
                        out=ex[:, 0:w], in_=ps[:, 0:w],
                        func=mybir.ActivationFunctionType.Exp, scale=SCALE,
                    )
                    if causal and sc * 128 > l0 - 128:
                        # keep where s0+p <= l0+off+f; base is always 0 here
                        # so only the leading 128 columns can be masked --
                        # beyond f=128 > p_max the predicate always holds.
                        # During the B-interleaved groups Pool is busy with
                        # k-rope, so mask there via the DVE triangle multiply
                        mw = min(w, 128 - (l0 + off - sc * 128))
                        if lqb <= 2:
                            nc.vector.tensor_mul(
                                ex[:, 0:mw], ex[:, 0:mw], tri_sb[:, 0:mw]
                            )
                        else:
                            nc.gpsimd.affine_select(
                                out=ex[:, 0:mw], in_=ex[:, 0:mw],
                                pattern=[[1, mw]],
                                compare_op=mybir.AluOpType.is_ge, fill=0.0,
                                base=l0 + off - sc * 128,
                                channel_multiplier=-1,
                            )
                    if not causal:
                        nc.vector.tensor_mul(ex, ex, mrows[:, sc, :])
                    # denominator accumulation on DVE; the first two full
                    # chunks fuse into a single 3-operand add
                    if sc == 0:
                        if n_sc == 1 or chunk_off(1) != 0:
                            nc.vector.tensor_copy(out=exsum, in_=ex)
                    elif sc == 1 and off == 0:
                        nc.vector.tensor_add(exsum, exps[0][0], ex)
                    else:
                        nc.vector.tensor_add(
                            exsum[:, off:512], exsum[:, off:512],
                            ex[:, 0:w],
                        )
                    exps.append((ex[:, 0:w], sc, off))
                    if sc >= DEPTH:
                        av_mm(*exps[sc - DEPTH])
                    if sc == 1:
                        run_fin_a()  # previous head's den sum + reciprocal
                    elif sc == max(2, min(4, n_sc - 1)):
                        run_fin_b()  # previous head's broadcast + normalize
                    if d_iter is not None and sc % d_every == d_every - 1:
                        emit_d_unit(d_iter, d_engs)
                for sc in range(max(0, n_sc - DEPTH), n_sc):
                    av_mm(*exps[sc])
                if n_sc <= 1:
                    run_fin_b()

                # denominator: one ones-matmul over the accumulated exsum
                # (vs one per chunk), reciprocal, then a PE partition
                # broadcast in f32r and the final normalize on DVE.
                # Deferred in two stages into the next head's chunk stream so
                # neither the scores-ring slots nor the DVE chain stall PE.
                fstate = {}

                def fin_a():
                    if DEN_MODE == "pool":
                        den = stats.tile([1, 512], F32, tag="den", bufs=2)
                        nc.gpsimd.tensor_reduce(
                            den, exsum, mybir.AxisListType.C,
                            mybir.AluOpType.add,
                        )
                    else:
                        den = psum.tile([1, 512], F32, tag="proj",
                                        bufs=3,
                                        name=f"pden_{qh}_{lqb}")
                        nc.tensor.matmul(den, ones_sb, exsum,
                                         start=True, stop=True)
                    rden = stats.tile([1, 512], F32R, tag="rden", bufs=2)
                    with nc.allow_low_precision(
                        reason="f32r reciprocal keeps full fp32 bits"
                    ):
                        nc.vector.reciprocal(out=rden, in_=den)
                    fstate["rden"] = rden

                def fin_b():
                    pbc = psum.tile([128, 512], F32, tag="scores",
                                    bufs=SCORES_BUFS, name=f"bc_{qh}_{lqb}")
                    nc.tensor.matmul(pbc, onesf_sb, fstate["rden"],
                                     start=True, stop=True)
                    rdenb = work.tile([128, 512], F32, tag="rdenb", bufs=2)
                    nc.vector.tensor_copy(out=rdenb, in_=pbc)
                    nc.vector.tensor_mul(
                        qkvT_sb[:, qh, l0:l0 + 512], pav, rdenb
                    )

                cstate["fin_a"] = fin_a
                cstate["fin_b"] = fin_b

            def c_block(lqb, d_iter=None, d_every=2,
                        d_engs=("vector", "scalar"), last_engs=None):
                for qh in range(nq):
                    eng = (last_engs if (last_engs and qh == nq - 1)
                           else d_engs)
                    c_head(lqb, qh, d_iter, d_every, eng)

            # ---- last output-projection group, hh-outer so only the final
            # 4 matmuls per wave wait on the last head's qkvT write ----
            d3_lbs = list(range(4 * (n_lqb - 1), n_lb))
            D3_TAGS = [("proj", 3), ("proj", 3), ("proj", 3),
                       ("scores", SCORES_BUFS)]

            def d3_wave_alloc(db):
                pos = {}
                for i, lb in enumerate(d3_lbs):
                    tg, bf = D3_TAGS[i % len(D3_TAGS)]
                    pos[lb] = psum.tile([128, 512], F32, tag=tg, bufs=bf,
                                        name=f"po3_{lb}_{db}")
                return pos

            def d3_mm(pos, db, hh_range):
                for hh in hh_range:
                    for lb in d3_lbs:
                        nc.tensor.matmul(
                            pos[lb],
                            qkvT_sb[:, hh, lb * 128:(lb + 1) * 128],
                            wo_sb[:, hh, db * 512:(db + 1) * 512],
                            start=(hh == 0), stop=(hh == nq - 1),
                            skip_group_check=True,
                        )

            def d3_finish_wave(pos, db):
                for i, lb in enumerate(d3_lbs):
                    oslab = outp.tile([128, 512], BF16, tag="ot3", bufs=6,
                                      name=f"ot3_{lb}_{db}")
                    _copy(nc, ("vector", "scalar")[i % 2], oslab, pos[lb])
                    nc.sync.dma_start(
                        out=out_d[lb * 128:(lb + 1) * 128,
                                  db * 512:(db + 1) * 512],
                        in_=oslab,
                    )

            def d3_fused(pre=None):
                for db in range(n_db):
                    if pre is not None and db == 0:
                        pos = pre
                        d3_mm(pos, db, range(nq - 1, nq))
                        d3_finish_wave(pos, db)
                    elif db == n_db - 1:
                        # two half-waves: the first half's copies and DMAs
                        # drain while the second half's matmuls still run,
                        # so only two units trail the final matmul
                        pos = d3_wave_alloc(db)
                        half = d3_lbs[:2]
                        for hh in range(nq):
                            for lb in half:
                                nc.tensor.matmul(
                                    pos[lb],
                                    qkvT_sb[:, hh, lb * 128:(lb + 1) * 128],
                                    wo_sb[:, hh, db * 512:(db + 1) * 512],
                                    start=(hh == 0), stop=(hh == nq - 1),
                                    skip_group_check=True,
                                )
                        for i, lb in enumerate(half):
                            oslab = outp.tile([128, 512], BF16, tag="ot3",
                                              bufs=6, name=f"ot3_{lb}_{db}")
                            _copy(nc, ("vector", "scalar")[i % 2],
                                  oslab, pos[lb])
                            nc.sync.dma_start(
                                out=out_d[lb * 128:(lb + 1) * 128,
                                          db * 512:(db + 1) * 512],
                                in_=oslab,
                            )
                        half = d3_lbs[2:]
                        for hh in range(nq):
                            for lb in half:
                                nc.tensor.matmul(
                                    pos[lb],
                                    qkvT_sb[:, hh, lb * 128:(lb + 1) * 128],
                                    wo_sb[:, hh, db * 512:(db + 1) * 512],
                                    start=(hh == 0), stop=(hh == nq - 1),
                                    skip_group_check=True,
                                )
                        for i, lb in enumerate(half):
                            oslab = outp.tile([128, 512], BF16, tag="ot3",
                                              bufs=6, name=f"ot3_{lb}_{db}")
                            _copy(nc, ("vector", "scalar")[i % 2],
                                  oslab, pos[lb])
                            nc.sync.dma_start(
                                out=out_d[lb * 128:(lb + 1) * 128,
                                          db * 512:(db + 1) * 512],
                                in_=oslab,
                            )
                    else:
                        pos = d3_wave_alloc(db)
                        d3_mm(pos, db, range(nq))
                        d3_finish_wave(pos, db)

            # ---- schedule ----
            if INTERLEAVE == "fine" and causal and n_lqb == 4 and n_lb == 16:
                xts = preload_startup()
                b_fused_start({k: v for k, v in xts.items() if k < 3})
                for lb in range(3, 9):
                    b_block(lb, xts.get(lb))
                if WO_LATE:
                    load_wo()
                del xts
                # C0/C1 head-units between the remaining B blocks: PE keeps
                # crunching proj matmuls while ACT chews the C exps.  The
                # last C1 heads (issued after every B block) already pull in
                # D0 units to plug PE bubbles from the exp backlog.
                it01 = d_iter_for([0, 1])
                cunits = ([(0, q, None) for q in range(nq)]
                          + [(1, 0, None), (1, 1, None),
                             (1, 2, it01), (1, 3, it01)])
                rest_b = list(range(9, n_lb))
                while cunits or rest_b:
                    if rest_b:
                        b_block(rest_b.pop(0))
                    if cunits:
                        lqb, qh, dit = cunits.pop(0)
                        c_head(lqb, qh, dit, d_every=2)
                # transposes for lb 13..15 stay pending: C2 needs only lb<=11;
                # pop them one per C2 head so their copies spread out
                for qh in range(nq):
                    c_head(2, qh, it01, d_every=2)
                    if state["pending"]:
                        transpose_block(*state["pending"].pop(0))
                flush_pending()
                drain_d(it01)
                it2 = d_iter_for([2])
                c_block(3, it2, d_every=4, d_engs=("vector",))
                # wave-0 partial sums for heads 0..2 keep PE busy while the
                # last head's denominator chain drains
                pre = d3_wave_alloc(0)
                d3_mm(pre, 0, range(nq - 1))
                run_fin_b()
                drain_d(it2, engs=("vector", "scalar"))
                d3_fused(pre)
            else:
                xts = preload_startup()
                for lb in range(n_lb):
                    b_block(lb, xts.get(lb))
                if WO_LATE:
                    load_wo()
                for g in range(n_lqb):
                    if g == n_lqb - 1 or not causal:
                        flush_pending()
                    c_block(g)
                    flush_pending()
                run_fin_b()
                drain_d(d_iter_for(list(range(n_lqb))))
    return nc


# ---------------- host side ----------------

def _x_block(xb, L_=L, D_=D):
    """Host-preblocked x: [lb, p, dc*128] with element x[lb*128+l, dc*128+p]."""
    n_lb, n_dc = L_ // 128, D_ // 128
    y = xb.reshape(n_lb, 128, n_dc, 128).transpose(0, 3, 2, 1)
    return np.ascontiguousarray(y).reshape(n_lb, 128, D_)


def _rope_block(pos, qw, kw, L_=L):
    """Host-preblocked rope tables: [n_grp, 128, grp*8*(H//2)] bf16 where
    element [gi, p, (Bi, a, j)] = table[a, (gi*grp+Bi)*128 + p, j]."""
    tabs = np.concatenate([_rope_tables(pos, qw), _rope_tables(pos, kw)])
    n_lb = L_ // 128
    grp = 4 if n_lb % 4 == 0 else 1
    t = tabs.reshape(8, n_lb // grp, grp, 128, H // 2)
    return np.ascontiguousarray(t.transpose(1, 3, 2, 0, 4)).reshape(
        n_lb // grp, 128, grp * 8 * (H // 2)
    ).astype(ml_dtypes.bfloat16)


def _rope_tables(pos, norm_w):
    """A,B,C,D [4, L, H/2] f32 with the rms-norm weight folded in.
    h1 = q1*A - q2*B ; h2 = q2*C + q1*D  (q already divided by rms)."""
    hh = H // 2
    fraction = 2.0 * np.arange(hh, dtype=np.float32) / np.float32(H)
    timescale = np.float32(ROPE_THETA) ** fraction
    sinusoid = pos.astype(np.float32)[:, None] / timescale[None, :]
    sin = np.sin(sinusoid).astype(np.float32)
    cos = np.cos(sinusoid).astype(np.float32)
    w1 = norm_w[:hh].astype(np.float32)
    w2 = norm_w[hh:].astype(np.float32)
    return np.stack([cos * w1, sin * w2, cos * w2, sin * w1]).astype(np.float32)


_KERNELS = {}
TRACE = False
LAST_RESULTS = None


def _get_kernel(causal):
    if causal not in _KERNELS:
        _KERNELS[causal] = build_core_kernel(causal=causal)
    return _KERNELS[causal]


def kernel(**inputs):
    x = np.asarray(inputs["x"], dtype=np.float32)
    pos = np.asarray(inputs["position_ids"])
    mask = np.asarray(inputs["attn_mask"]).astype(bool)
    wq = np.asarray(inputs["wq"], dtype=np.float32)
    wk = np.asarray(inputs["wk"], dtype=np.float32)
    wv = np.asarray(inputs["wv"], dtype=np.float32)
    wo = np.asarray(inputs["wo"], dtype=np.float32)
    qw = np.asarray(inputs["q_norm_w"], dtype=np.float32)
    kw = np.asarray(inputs["k_norm_w"], dtype=np.float32)

    tril = np.tril(np.ones((L, L), dtype=bool))
    causal = all(np.array_equal(mask[b], tril) for b in range(B))
    nc = _get_kernel(causal)

    bf = ml_dtypes.bfloat16
    per_batch = []
    for b in range(B):
        d = {
            "xT": _x_block(x[b].astype(bf)),
            "rope": _rope_block(pos[b], qw, kw),
        }
        if not causal:
            d["maskT"] = np.ascontiguousarray(mask[b].T).astype(bf)
        per_batch.append(d)

    in_maps = []
    for c in range(N_CORES):
        b, g = divmod(c, N_CORES // B)
        qs = slice(QH_PER_CORE * g, QH_PER_CORE * (g + 1))
        ks = slice(KV_PER_CORE * g, KV_PER_CORE * (g + 1))
        wqkv = np.concatenate(
            [
                wq[:, qs, :].reshape(D, QH_PER_CORE * H),
                wk[:, ks, :].reshape(D, KV_PER_CORE * H),
                wv[:, ks, :].reshape(D, KV_PER_CORE * H),
            ],
            axis=1,
        ).astype(bf)
        m = dict(per_batch[b])
        m["wqkv"] = wqkv
        m["wo"] = np.ascontiguousarray(wo[qs].reshape(QH_PER_CORE * H, D)).astype(bf)
        in_maps.append(m)

    global LAST_RESULTS
    res = run_bass_kernel_spmd(
        nc, in_maps, core_ids=list(range(N_CORES)), trace=TRACE
    )
    LAST_RESULTS = res
    out = np.zeros((B, L, D), dtype=np.float32)
    for c in range(N_CORES):
        out[c // (N_CORES // B)] += res.results[c]["out"]
    return out

